# revision 1
# baseline (speedup 1.0000x reference)
"""Complex LSTM cell (CLSTMCell) Trainium2 kernel.

Full inputs in, full outputs out. Data-parallel over batch: B=4096 rows
sharded 512/core across 8 NeuronCores; the 8 complex weight matrices are
replicated (host pre-packed into a matmul-friendly layout).

Math: with X1=[xr|hr], X2=[xi|hi] ([B,2048]) and W1=[Ur;Wr], W2=[Ui;Wi]
([2048,4096]), the complex gate projection is computed via Karatsuba:
  P1 = X1@W1, P2 = X2@W2, P3 = (X1+X2)@(W1+W2)
  Zr = P1 - P2 (+ br),  Zi = P3 - P1 - P2 (+ bi)
i.e. 3 real matmuls instead of 4 (25% FLOP cut). Matmuls run in bf16
(fp32 PSUM accumulation); the elementwise gate epilogue runs in fp32.

Weight columns are interleaved as c = oblk*512 + gate*128 + (o % 128)
so each N=512 matmul block contains all 4 gates for one 128-wide o
slice, letting the cell update complete per-block with no cross-block
buffering.
"""

import sys

for _p in ("/opt/trn_rl_repo",):
    if _p not in sys.path:
        sys.path.insert(0, _p)

import numpy as np
import ml_dtypes

import concourse.bass as bass
import concourse.mybir as mybir
from concourse.bass_utils import run_bass_kernel_spmd
from concourse.tile import TileContext, add_dep_helper

F32 = mybir.dt.float32
BF16 = mybir.dt.bfloat16
AFT = mybir.ActivationFunctionType

B = 4096
IN = 1024
H = 1024
G = 4
NCORES = 8
BL = B // NCORES          # 512 batch rows per core
MT = BL // 128            # 4 m-tiles per core
K = 2 * IN                # 2048 contraction dim (x|h concat)
KT = K // 128             # 16 k-tiles
OB = H // 128             # 8 o-blocks
NW = G * 128              # 512 matmul N (all gates for one o-block)


def _split_multiwait_json(raw: bytes) -> bytes:
    """The walrus build in this container accepts at most one sem wait
    per instruction; Tile's scheduler packs several. Split the extras
    into preceding wait-only EventSemaphore instructions on the same
    engine (same semantics: the sequencer blocks on each in order)."""
    import orjson

    m = orjson.loads(raw)
    ctr = 0
    for fn in m["functions"]:
        for bb in fn["blocks"]:
            out = []
            for ins in bb["instructions"]:
                si = ins.get("sync_info")
                waits = si.get("on_wait") if si else None
                if waits and len(waits) > 1:
                    for w in waits[:-1]:
                        ctr += 1
                        nop = {
                            "engine": ins["engine"],
                            "ins": [],
                            "outs": [],
                            "name": f"{ins['name']}_sw{ctr}",
                            "opcode": "EventSemaphore",
                            "sync_info": {"on_update": [], "on_wait": [w]},
                        }
                        if "debug" in ins:
                            nop["debug"] = ins["debug"]
                        out.append(nop)
                    si["on_wait"] = [waits[-1]]
                out.append(ins)
            bb["instructions"] = out
    return orjson.dumps(m)


def _build_program(repeat=1, timing=False):
    # timing=True builds a NEFF with Internal (unshipped) I/O and the main
    # loop emitted `repeat` times — used only to measure per-step device
    # time without the host<->device transfer cost dominating.
    nc = bass.Bass()

    kin = "Internal" if timing else "ExternalInput"
    kout = "Internal" if timing else "ExternalOutput"
    x1tp = nc.dram_tensor("x1tp", [KT, 128, BL], BF16, kind=kin)
    x2tp = nc.dram_tensor("x2tp", [KT, 128, BL], BF16, kind=kin)
    cx = nc.dram_tensor("cx", [BL, 2 * H], BF16, kind=kin)
    wpk = nc.dram_tensor("wpk", [3, OB, 128, KT, NW], BF16, kind=kin)
    bbc = nc.dram_tensor("bbc", [2, 128, G * H], BF16, kind="ExternalInput")
    h_out = nc.dram_tensor("h_out", [BL, 2 * H], F32, kind=kout)
    c_out = nc.dram_tensor("c_out", [BL, 2 * H], F32, kind=kout)
    sink = (
        nc.dram_tensor("sink", [1, 4], F32, kind="ExternalOutput")
        if timing
        else None
    )

    with TileContext(nc) as tc:
        with (
            tc.tile_pool(name="const", bufs=1) as constp,
            tc.tile_pool(name="cres", bufs=1) as cresp,
            tc.tile_pool(name="xt", bufs=1) as xtp,
            tc.tile_pool(name="w", bufs=3) as wp,
            tc.tile_pool(name="pp", bufs=1) as pp,
            tc.tile_pool(name="ep", bufs=2) as epp,
            tc.tile_pool(name="prod", bufs=2) as prodp,
            tc.tile_pool(name="ps_mm", bufs=6, space="PSUM") as psmm,
        ):
            # bias rows land as two 8KB DMAs and are partition-broadcast
            # in place by GPSIMD — keeps the SP HWDGE ring free for the
            # first weight slab at kernel start.
            bias_r = constp.tile([128, G * H], BF16, tag="bias_r", name="bias_r")
            bias_i = constp.tile([128, G * H], BF16, tag="bias_i", name="bias_i")
            bias_dma_holder = []

            ctile = []

            # X1.T / X2.T k-tiles in bf16 via cast-DMA + PE transpose;
            # X3.T = X1.T + X2.T on DVE.
            x1t = [xtp.tile([128, BL], BF16, tag=f"x1t_{k}", name=f"x1t_{k}") for k in range(KT)]
            x2t = [xtp.tile([128, BL], BF16, tag=f"x2t_{k}", name=f"x2t_{k}") for k in range(KT)]
            x3t = [xtp.tile([128, BL], BF16, tag=f"x3t_{k}", name=f"x3t_{k}") for k in range(KT)]
            def emit_transposes(xtsrc, dst):
                # whole X.T pre-packed on host: straight k-tile DMA loads
                dmas = []
                for k in range(KT):
                    dmas.append(nc.gpsimd.dma_start(out=dst[k][:], in_=xtsrc[k]))
                return dmas

            xts = (x1t, x2t, x3t)

            def cmul(outr, outi, ar, ai, br, bi, pfx):
                """(outr + i*outi) = (ar + i*ai) * (br + i*bi), fp32 DVE."""
                t1 = prodp.tile([128, 128], F32, tag=f"{pfx}1", name=f"{pfx}1")
                t2 = prodp.tile([128, 128], F32, tag=f"{pfx}2", name=f"{pfx}2")
                t3 = prodp.tile([128, 128], F32, tag=f"{pfx}3", name=f"{pfx}3")
                t4 = prodp.tile([128, 128], F32, tag=f"{pfx}4", name=f"{pfx}4")
                nc.vector.tensor_mul(t1[:], ar, br)
                nc.vector.tensor_mul(t2[:], ai, bi)
                nc.vector.tensor_mul(t3[:], ar, bi)
                nc.vector.tensor_mul(t4[:], ai, br)
                nc.vector.tensor_sub(outr, t1[:], t2[:])
                nc.vector.tensor_add(outi, t3[:], t4[:])

            p1s_all = {ob: [None] * MT for ob in range(OB)}
            p2s_all = {ob: [None] * MT for ob in range(OB)}
            pa = [None] * MT

            def emit_mat(ob, mat):
                ocols = slice(ob * 128, (ob + 1) * 128)
                oicols = slice(H + ob * 128, H + (ob + 1) * 128)
                p1s = p1s_all[ob]
                p2s = p2s_all[ob]
                if True:
                    w = wp.tile([128, KT * NW], BF16, tag="wslab", name="wslab")
                    wsrc = wpk[mat, ob].rearrange("p kt c -> p (kt c)")
                    if ob == 0 and mat == 0:
                        # split the very first slab so matmuls start as
                        # quarters land
                        qr = KT * NW // 4
                        for _q in range(4):
                            nc.sync.dma_start(
                                out=w[:, _q * qr : (_q + 1) * qr],
                                in_=wsrc[:, _q * qr : (_q + 1) * qr],
                            )
                    elif ob == 0 and mat == 1:
                        half = KT * NW // 2
                        nc.sync.dma_start(out=w[:, :half], in_=wsrc[:, :half])
                        nc.sync.dma_start(out=w[:, half:], in_=wsrc[:, half:])
                    else:
                        nc.sync.dma_start(out=w[:], in_=wsrc)
                    for m in range(MT):
                        rows = slice(m * 128, (m + 1) * 128)
                        ps = psmm.tile([128, NW], F32, tag="mm", name="mm")
                        for k in range(KT):
                            nc.tensor.matmul(
                                ps[:],
                                lhsT=xts[mat][k][:, rows],
                                rhs=w[:, k * NW : (k + 1) * NW],
                                start=(k == 0),
                                stop=(k == KT - 1),
                            )
                        if mat == 0:
                            p1s[m] = pp.tile([128, NW], F32, tag=f"p1_{m}", name=f"p1_{m}")
                            nc.scalar.copy(p1s[m][:], ps[:])
                        elif mat == 1:
                            p2s[m] = pp.tile([128, NW], F32, tag=f"p2_{m}", name=f"p2_{m}")
                            nc.scalar.copy(p2s[m][:], ps[:])
                            # ---- epilogue phase A: everything that only
                            # needs P1/P2 (not P3) — overlaps the P3 matmuls.
                            obw = slice(ob * NW, (ob + 1) * NW)
                            p1, p2 = p1s[m], p2s[m]
                            zr = epp.tile([128, NW], F32, tag="zra", name="zra")
                            nc.vector.tensor_sub(zr[:], p1[:], p2[:])
                            nc.gpsimd.tensor_add(zr[:], zr[:], bias_r[:, obw])
                            gr = epp.tile([128, NW], F32, tag=f"gr_{m}", name=f"gr_{m}", bufs=1)
                            nc.scalar.activation(gr[:, 0:384], zr[:, 0:384], AFT.Sigmoid)
                            nc.scalar.activation(gr[:, 384:512], zr[:, 384:512], AFT.Tanh)
                            # q = p1 + p2 - bias_i: lets phase B produce
                            # zi = P3 - q in a single DVE op off the bias path
                            q = epp.tile([128, NW], F32, tag=f"q_{m}", name=f"q_{m}", bufs=1)
                            nc.vector.tensor_add(q[:], p1[:], p2[:])
                            nc.gpsimd.tensor_sub(q[:], q[:], bias_i[:, obw])
                            cr = ctile[m][:, ocols]
                            ci = ctile[m][:, oicols]
                            fr = gr[:, 0:128]
                            ir_ = gr[:, 128:256]
                            ar = gr[:, 384:512]
                            u1 = prodp.tile([128, 128], F32, tag=f"u1_{m}", name=f"u1_{m}", bufs=1)
                            u4 = prodp.tile([128, 128], F32, tag=f"u4_{m}", name=f"u4_{m}", bufs=1)
                            v1 = prodp.tile([128, 128], F32, tag=f"v1_{m}", name=f"v1_{m}", bufs=1)
                            nc.vector.tensor_mul(u1[:], cr, fr)
                            nc.vector.tensor_mul(u4[:], ci, fr)
                            nc.vector.tensor_mul(v1[:], ar, ir_)
                            pa[m] = (gr, q, u1, u4, v1)
                        else:
                            # ---- epilogue phase B for (ob, m): ps holds P3 ----
                            obw = slice(ob * NW, (ob + 1) * NW)
                            gr, q, u1, u4, v1 = pa[m]
                            zi = epp.tile([128, NW], F32, tag="zi", name="zi")
                            # halves: the sigmoid can start after the first
                            # half-subtract instead of the full-width op
                            nc.vector.tensor_sub(zi[:, 0:256], ps[:, 0:256], q[:, 0:256])
                            nc.vector.tensor_sub(zi[:, 256:512], ps[:, 256:512], q[:, 256:512])
                            gi = epp.tile([128, NW], F32, tag="gi", name="gi")
                            nc.scalar.activation(gi[:, 0:256], zi[:, 0:256], AFT.Sigmoid)
                            nc.scalar.activation(gi[:, 256:384], zi[:, 256:384], AFT.Sigmoid)
                            nc.scalar.activation(gi[:, 384:512], zi[:, 384:512], AFT.Tanh)
                            cr = ctile[m][:, ocols]
                            ci = ctile[m][:, oicols]
                            fi = gi[:, 0:128]
                            ii_ = gi[:, 128:256]
                            oi = gi[:, 256:384]
                            ai = gi[:, 384:512]
                            ir_ = gr[:, 128:256]
                            orr = gr[:, 256:384]
                            ar = gr[:, 384:512]
                            u2 = prodp.tile([128, 128], F32, tag="u2", name="u2")
                            u3 = prodp.tile([128, 128], F32, tag="u3", name="u3")
                            v2 = prodp.tile([128, 128], F32, tag="v2", name="v2")
                            v3 = prodp.tile([128, 128], F32, tag="v3", name="v3")
                            v4 = prodp.tile([128, 128], F32, tag="v4", name="v4")
                            nc.vector.tensor_mul(u2[:], ci, fi)
                            nc.vector.tensor_mul(u3[:], cr, fi)
                            nc.gpsimd.tensor_mul(v2[:], ai, ii_)
                            nc.gpsimd.tensor_mul(v3[:], ar, ii_)
                            nc.vector.tensor_mul(v4[:], ai, ir_)
                            cfr = prodp.tile([128, 128], F32, tag="cfr", name="cfr")
                            cfi = prodp.tile([128, 128], F32, tag="cfi", name="cfi")
                            air = prodp.tile([128, 128], F32, tag="air", name="air")
                            aii = prodp.tile([128, 128], F32, tag="aii", name="aii")
                            nc.vector.tensor_sub(cfr[:], u1[:], u2[:])
                            nc.vector.tensor_add(cfi[:], u3[:], u4[:])
                            nc.gpsimd.tensor_sub(air[:], v1[:], v2[:])
                            nc.gpsimd.tensor_add(aii[:], v3[:], v4[:])
                            ctr = prodp.tile([128, 128], F32, tag="ctr", name="ctr")
                            cti = prodp.tile([128, 128], F32, tag="cti", name="cti")
                            nc.vector.tensor_add(ctr[:], cfr[:], air[:])
                            nc.vector.tensor_add(cti[:], cfi[:], aii[:])
                            tr = prodp.tile([128, 128], F32, tag="tr", name="tr")
                            ti = prodp.tile([128, 128], F32, tag="ti", name="ti")
                            nc.scalar.activation(tr[:], ctr[:], AFT.Tanh)
                            nc.scalar.activation(ti[:], cti[:], AFT.Tanh)
                            htr = prodp.tile([128, 128], F32, tag="htr", name="htr")
                            hti = prodp.tile([128, 128], F32, tag="hti", name="hti")
                            w1 = prodp.tile([128, 128], F32, tag="w1", name="w1")
                            w2 = prodp.tile([128, 128], F32, tag="w2", name="w2")
                            w3 = prodp.tile([128, 128], F32, tag="w3", name="w3")
                            w4 = prodp.tile([128, 128], F32, tag="w4", name="w4")
                            # real half on DVE, imag half on GPSIMD in parallel
                            nc.vector.tensor_mul(w1[:], orr, tr[:])
                            nc.vector.tensor_mul(w2[:], oi, ti[:])
                            nc.vector.tensor_sub(htr[:], w1[:], w2[:])
                            nc.gpsimd.tensor_mul(w3[:], orr, ti[:])
                            nc.gpsimd.tensor_mul(w4[:], oi, tr[:])
                            nc.gpsimd.tensor_add(hti[:], w3[:], w4[:])
                            nc.sync.dma_start(out=h_out[rows, ocols], in_=htr[:])
                            nc.scalar.dma_start(out=h_out[rows, oicols], in_=hti[:])
                            nc.scalar.dma_start(out=c_out[rows, ocols], in_=ctr[:])
                            nc.sync.dma_start(out=c_out[rows, oicols], in_=cti[:])


            # PE stream order: x1 transposes, then the first matmul block
            # (only needs X1.T), then x2 transposes + x3 adds while that
            # block runs, then everything else.
            d1 = emit_transposes(x1tp, x1t)
            if repeat > 0:
                emit_mat(0, 0)
            d2 = emit_transposes(x2tp, x2t)
            # host-broadcast bias tiles ride the SWDGE ring after all
            # x/h loads (first use is the phase-A gpsimd adds at ~t=30us)
            for _bt, _bi in ((bias_r, 0), (bias_i, 1)):
                d = nc.gpsimd.dma_start(out=_bt[:], in_=bbc[_bi])
                add_dep_helper(d.ins, d2[-1].ins, sync=False,
                               reason="bias loads after x/h loads")
            # c tiles ride the SWDGE ring after all x/h loads (first use of
            # c is the phase-A products at ~t=30us)
            for m in range(MT):
                t = cresp.tile([128, 2 * H], BF16, tag=f"c_m{m}", name=f"c_m{m}")
                d = nc.gpsimd.dma_start(out=t[:], in_=cx[m * 128 : (m + 1) * 128, :])
                add_dep_helper(d.ins, d2[-1].ins, sync=False,
                               reason="c loads after x/h loads")
                ctile.append(t)
            for k in range(KT):
                nc.vector.tensor_add(x3t[k][:], x1t[k][:], x2t[k][:])
            if repeat > 0:
                emit_mat(0, 1)
                emit_mat(0, 2)
                for ob in range(1, OB):
                    for mat in range(3):
                        emit_mat(ob, mat)
            for _rep in range(1, repeat):
                for ob in range(OB):
                    for mat in range(3):
                        emit_mat(ob, mat)
            if sink is not None:
                nc.gpsimd.dma_start(out=sink[:], in_=bias_r[0:1, 0:4])
    return nc


_NC_CACHE = None


def _get_program():
    global _NC_CACHE
    if _NC_CACHE is None:
        nc = _build_program()
        fixed = _split_multiwait_json(nc.to_json_bytes())
        nc.to_json_bytes = lambda: fixed
        _NC_CACHE = nc
    return _NC_CACHE


def _pack_weights(Uw_r, Uw_i, Ww_r, Ww_i, Ub_r, Ub_i, Wb_r, Wb_i):
    GORD = [0, 1, 3, 2]  # gate order f, i, o, a within each o-block:
    # the three sigmoid gates are contiguous so the epilogue needs one
    # sigmoid call [0:384] and one tanh call [384:512] per z tile.

    def interleave_cols(Wg):  # [2048, G, H] -> [2048, GH], c = ob*512+gidx*128+oi
        return (
            Wg.reshape(K, G, OB, 128)[:, GORD]
            .transpose(0, 2, 1, 3)
            .reshape(K, G * H)
        )

    # [G, H(o), D(k)] -> [k, G, o], stack x-side over h-side along k
    Wr = np.concatenate(
        [np.transpose(Uw_r, (2, 0, 1)), np.transpose(Ww_r, (2, 0, 1))], axis=0
    )
    Wi = np.concatenate(
        [np.transpose(Uw_i, (2, 0, 1)), np.transpose(Ww_i, (2, 0, 1))], axis=0
    )
    W1 = interleave_cols(Wr)
    W2 = interleave_cols(Wi)
    W3 = W1 + W2
    Wall = np.stack([W1, W2, W3])  # [3, 2048, 4096] fp32
    # -> [3, ob, p(128), kt, c(512)]: per (mat, ob) this is exactly the
    # SBUF slab layout [128 partitions x (kt*512) free], so the weight
    # DMA is one contiguous 2 MB copy.
    wpk = (
        Wall.reshape(3, KT, 128, OB, NW)
        .transpose(0, 3, 2, 1, 4)
        .astype(ml_dtypes.bfloat16)
    )

    def interleave_bias(b):  # [G, H] -> [GH] interleaved
        return b.reshape(G, OB, 128)[GORD].transpose(1, 0, 2).reshape(G * H)

    br = interleave_bias(Ub_r + Wb_r)
    bi = interleave_bias(Ub_i + Wb_i)
    bbc = np.ascontiguousarray(np.broadcast_to(
        np.stack([br, bi])[:, None, :], (2, 128, G * H)
    ).astype(ml_dtypes.bfloat16))
    return np.ascontiguousarray(wpk), np.ascontiguousarray(bbc)


def kernel(input, h_x, c_x, Uw_r, Uw_i, Ub_r, Ub_i, Ww_r, Ww_i, Wb_r, Wb_i,
           _trace=False):
    input = np.asarray(input, dtype=np.float32)
    h_x = np.asarray(h_x, dtype=np.float32)
    c_x = np.asarray(c_x, dtype=np.float32)
    wpk, bpk = _pack_weights(
        np.asarray(Uw_r, np.float32), np.asarray(Uw_i, np.float32),
        np.asarray(Ww_r, np.float32), np.asarray(Ww_i, np.float32),
        np.asarray(Ub_r, np.float32), np.asarray(Ub_i, np.float32),
        np.asarray(Wb_r, np.float32), np.asarray(Wb_i, np.float32),
    )

    x1b = np.concatenate([input[:, :IN], h_x[:, :H]], axis=1).astype(ml_dtypes.bfloat16)
    x2b = np.concatenate([input[:, IN:], h_x[:, H:]], axis=1).astype(ml_dtypes.bfloat16)

    in_maps = []
    for c in range(NCORES):
        rows = slice(c * BL, (c + 1) * BL)
        in_maps.append(
            {
                "x1tp": np.ascontiguousarray(
                    x1b[rows].T.reshape(KT, 128, BL)
                ),
                "x2tp": np.ascontiguousarray(
                    x2b[rows].T.reshape(KT, 128, BL)
                ),
                "cx": np.ascontiguousarray(c_x[rows].astype(ml_dtypes.bfloat16)),
                "wpk": wpk,
                "bbc": bpk,
            }
        )

    nc = _get_program()
    res = run_bass_kernel_spmd(
        nc, in_maps, core_ids=list(range(NCORES)), trace=_trace
    )
    h_t = np.concatenate([res.results[i]["h_out"] for i in range(NCORES)], axis=0)
    c_t = np.concatenate([res.results[i]["c_out"] for i in range(NCORES)], axis=0)
    if _trace:
        kernel._last_results = res
    return h_t, c_t



# revision 6
# speedup vs baseline: 1.4675x; 1.4675x over previous
"""Complex LSTM cell (CLSTMCell) Trainium2 kernel — fp8 DoubleRow edition.

Full inputs in, full outputs out. Data-parallel over batch: B=4096 rows
sharded 512/core across 8 NeuronCores; the weight matrices are replicated
(host pre-packed into a matmul-friendly fp8 layout).

Math: with X1=[xr|hr], X2=[xi|hi] ([B,2048]) and W1=[Ur;Wr], W2=[Ui;Wi]
([2048,4096]), the complex gate projection is computed via Karatsuba:
  P1 = X1@W1, P2 = X2@W2, P3 = (X1+X2)@(W1+W2)
  Zr = P1 - P2 (+ br),  Zi = P3 - P1 - P2 (+ bi)
i.e. 3 real matmuls instead of 4 (25% FLOP cut).

Matmuls run in fp8-e4m3 with MatmulPerfMode.DoubleRow (two k-subtiles per
instruction at 0.5 cycles/row = 4x bf16 PE throughput). fp8's 3-bit
mantissa alone is too coarse (rel err ~5e-2 > 2e-2 gate), so each operand
is split hi/lo: X = Xh + Xl, W*64 = Wh + Wl (all four parts fp8, common
product scale 64 folded out via the activation `scale` arg). The product
is corrected per gate-column block:
  P = Xh@Wh [+ Xl@Wh (X-corr)] [+ Xh@Wl (W-corr)]
Correction column sets are per-mat suffixes of the gate order [o,f,i,a]
(CORR_CFG below), chosen by offline error search: sigmoid gates tolerate
~4x more z-error than what full-correction provides, so only the
sensitive columns pay the extra DR passes.

Weight columns are interleaved as c = oblk*512 + gate*128 + (o % 128)
with gate order [o,f,i,a], so each N=512 matmul block contains all 4
gates for one 128-wide o slice, letting the cell update complete
per-block with no cross-block buffering.
"""

import sys

for _p in ("/opt/trn_rl_repo",):
    if _p not in sys.path:
        sys.path.insert(0, _p)

import numpy as np
import ml_dtypes

import concourse.bass as bass
import concourse.mybir as mybir
from concourse.bass_utils import run_bass_kernel_spmd
from concourse.tile import TileContext, add_dep_helper

F32 = mybir.dt.float32
BF16 = mybir.dt.bfloat16
F8 = mybir.dt.float8e4
AFT = mybir.ActivationFunctionType
DR = mybir.MatmulPerfMode.DoubleRow

B = 4096
IN = 1024
H = 1024
G = 4
NCORES = 8
BL = B // NCORES          # 512 batch rows per core
MT = BL // 128            # 4 m-tiles per core
K = 2 * IN                # 2048 contraction dim (x|h concat)
KT = K // 128             # 16 k-tiles
OB = H // 8 // 128 * 8    # placeholder; real OB below
OB = H // 128             # 8 o-blocks
NW = G * 128              # 512 matmul N (all gates for one o-block)
SX = 4.0                  # x-side pre-scale
SW = 1024.0               # weight pre-scale
SP = SX * SW              # product scale, folded out via activation scale
# Scales keep all four fp8 operand classes (Xh, Xl, Wh, Wl) out of e4m3's
# subnormal range (hi parts sigma ~4 / ~22; residuals sigma ~0.07 / ~0.4,
# vs tiny=0.0156), so correctness survives even if the PE flushes fp8
# subnormals (the interpreter doesn't, hardware behavior unverified).

# Gate order within each 512-wide o-block: [i, f, o, a] (measured output
# sensitivity to z-error: a > o > f > i).
# Column slices: i=[0:128] f=[128:256] o=[256:384] a=[384:512].
# Correction sets are suffixes [start:512]; per mat (P1, P2, P3):
#   (xcorr_start, wcorr_start), 512 = no correction of that side.
# Offline greedy search: skipping i-gate corrections on P1/P2 costs
# +1.0e-2 rel err (total ~1.2e-2 vs 2e-2 gate) and saves 0.25 units.
CORR_CFG = [(128, 128), (128, 128), (0, 0)]


def _split_multiwait_json(raw: bytes) -> bytes:
    """The walrus build in this container accepts at most one sem wait
    per instruction; Tile's scheduler packs several. Split the extras
    into preceding wait-only EventSemaphore instructions on the same
    engine (same semantics: the sequencer blocks on each in order)."""
    import orjson

    m = orjson.loads(raw)
    ctr = 0
    for fn in m["functions"]:
        for bb in fn["blocks"]:
            out = []
            for ins in bb["instructions"]:
                si = ins.get("sync_info")
                waits = si.get("on_wait") if si else None
                if waits and len(waits) > 1:
                    for w in waits[:-1]:
                        ctr += 1
                        nop = {
                            "engine": ins["engine"],
                            "ins": [],
                            "outs": [],
                            "name": f"{ins['name']}_sw{ctr}",
                            "opcode": "EventSemaphore",
                            "sync_info": {"on_update": [], "on_wait": [w]},
                        }
                        if "debug" in ins:
                            nop["debug"] = ins["debug"]
                        out.append(nop)
                    si["on_wait"] = [waits[-1]]
                out.append(ins)
            bb["instructions"] = out
    return orjson.dumps(m)


def _build_program():
    nc = bass.Bass()

    # x-side: hi/lo fp8 for X1, X2, X3, pre-transposed on host: [KT,128,BL]
    xh = [nc.dram_tensor(f"x{m}h", [KT, 128, BL], F8, kind="ExternalInput")
          for m in range(3)]
    xl = [nc.dram_tensor(f"x{m}l", [KT, 128, BL], F8, kind="ExternalInput")
          for m in range(3)]
    cx = nc.dram_tensor("cx", [BL, 2 * H], BF16, kind="ExternalInput")
    # hi slabs: [3, OB, 128, KT, NW]; lo slabs only for corrected columns
    wh = nc.dram_tensor("wh", [3, OB, 128, KT, NW], F8, kind="ExternalInput")
    wlw = [NW - s for s, _ in [(c[1], 0) for c in CORR_CFG]]
    wl = [
        nc.dram_tensor(f"wl{m}", [OB, 128, KT, NW - CORR_CFG[m][1]], F8,
                       kind="ExternalInput")
        if CORR_CFG[m][1] < NW else None
        for m in range(3)
    ]
    bbc = nc.dram_tensor("bbc", [2, 128, G * H], BF16, kind="ExternalInput")
    h_out = nc.dram_tensor("h_out", [BL, 2 * H], BF16, kind="ExternalOutput")
    c_out = nc.dram_tensor("c_out", [BL, 2 * H], BF16, kind="ExternalOutput")

    with TileContext(nc) as tc:
        with (
            tc.tile_pool(name="const", bufs=1) as constp,
            tc.tile_pool(name="cres", bufs=1) as cresp,
            tc.tile_pool(name="xt", bufs=1) as xtp,
            tc.tile_pool(name="w", bufs=3) as wp,
            tc.tile_pool(name="wlp", bufs=3) as wlp,
            tc.tile_pool(name="pp", bufs=1) as pp,
            tc.tile_pool(name="ep", bufs=2) as epp,
            tc.tile_pool(name="prod", bufs=2) as prodp,
            tc.tile_pool(name="ps_mm", bufs=6, space="PSUM") as psmm,
        ):
            bias_r = constp.tile([128, G * H], BF16, tag="bias_r", name="bias_r")
            bias_i = constp.tile([128, G * H], BF16, tag="bias_i", name="bias_i")

            ctile = []

            # X tiles: one [128, KT, BL] tile per tensor; DR slices
            # [:, 2k:2k+2, m*128:(m+1)*128] need k-pairs adjacent in dim1.
            xht = [xtp.tile([128, KT, BL], F8, tag=f"xh{m}", name=f"xh{m}")
                   for m in range(3)]
            xlt = [
                xtp.tile([128, KT, BL], F8, tag=f"xl{m}", name=f"xl{m}")
                if CORR_CFG[m][0] < NW else None
                for m in range(3)
            ]

            def emit_xloads(src, dst, quarters=4):
                # dram [KT,128,BL] -> sbuf [128, KT, BL] in `quarters` DMAs
                dmas = []
                kq = KT // quarters
                for q in range(quarters):
                    dmas.append(nc.gpsimd.dma_start(
                        out=dst[:, q * kq : (q + 1) * kq, :],
                        in_=src[q * kq : (q + 1) * kq].rearrange(
                            "kt p b -> p kt b"
                        ),
                    ))
                return dmas

            p1s_all = {ob: [None] * MT for ob in range(OB)}
            p2s_all = {ob: [None] * MT for ob in range(OB)}
            pa = [None] * MT

            def emit_mat(ob, mat):
                ocols = slice(ob * 128, (ob + 1) * 128)
                oicols = slice(H + ob * 128, H + (ob + 1) * 128)
                p1s = p1s_all[ob]
                p2s = p2s_all[ob]
                xc0, wc0 = CORR_CFG[mat]
                w = wp.tile([128, KT, NW], F8, tag="wslab", name="wslab")
                wsrc = wh[mat, ob].rearrange("p kt c -> p (kt c)")
                wdst = w.rearrange("p kt c -> p (kt c)")
                if ob == 0 and mat == 0:
                    # split the very first slab so matmuls start as
                    # quarters land
                    qr = KT * NW // 4
                    for _q in range(4):
                        nc.sync.dma_start(
                            out=wdst[:, _q * qr : (_q + 1) * qr],
                            in_=wsrc[:, _q * qr : (_q + 1) * qr],
                        )
                elif ob == 0 and mat == 1:
                    half = KT * NW // 2
                    nc.sync.dma_start(out=wdst[:, :half], in_=wsrc[:, :half])
                    nc.sync.dma_start(out=wdst[:, half:], in_=wsrc[:, half:])
                else:
                    nc.sync.dma_start(out=wdst[:], in_=wsrc)
                wlt = None
                if wc0 < NW:
                    ncw = NW - wc0
                    wlt = wlp.tile([128, KT, ncw], F8, tag="wlslab",
                                   name="wlslab")
                    nc.sync.dma_start(
                        out=wlt.rearrange("p kt c -> p (kt c)"),
                        in_=wl[mat][ob].rearrange("p kt c -> p (kt c)"),
                    )
                for m in range(MT):
                    rows = slice(m * 128, (m + 1) * 128)
                    ps = psmm.tile([128, NW], F32, tag="mm", name="mm")
                    last_pass = "wc" if wc0 < NW else ("xc" if xc0 < NW else "main")
                    # main pass: Xh @ Wh, full width
                    for k in range(0, KT, 2):
                        nc.tensor.matmul(
                            ps[:],
                            lhsT=xht[mat][:, k : k + 2, rows],
                            rhs=w[:, k : k + 2, :],
                            start=(k == 0),
                            stop=(last_pass == "main" and k == KT - 2),
                            perf_mode=DR,
                        )
                    # X-corr pass: Xl @ Wh on columns [xc0:512]
                    if xc0 < NW:
                        for k in range(0, KT, 2):
                            nc.tensor.matmul(
                                ps[:, xc0:NW],
                                lhsT=xlt[mat][:, k : k + 2, rows],
                                rhs=w[:, k : k + 2, xc0:NW],
                                start=False,
                                stop=(last_pass == "xc" and k == KT - 2),
                                perf_mode=DR,
                            )
                    # W-corr pass: Xh @ Wl on columns [wc0:512]
                    if wc0 < NW:
                        for k in range(0, KT, 2):
                            nc.tensor.matmul(
                                ps[:, wc0:NW],
                                lhsT=xht[mat][:, k : k + 2, rows],
                                rhs=wlt[:, k : k + 2, :],
                                start=False,
                                stop=(last_pass == "wc" and k == KT - 2),
                                perf_mode=DR,
                            )

                    if mat == 0:
                        p1s[m] = pp.tile([128, NW], F32, tag=f"p1_{m}",
                                         name=f"p1_{m}")
                        nc.scalar.copy(p1s[m][:], ps[:])
                    elif mat == 1:
                        p2s[m] = pp.tile([128, NW], F32, tag=f"p2_{m}",
                                         name=f"p2_{m}")
                        nc.scalar.copy(p2s[m][:], ps[:])
                        # ---- epilogue phase A: everything that only
                        # needs P1/P2 (not P3) — overlaps the P3 matmuls.
                        # Gate cols: o=[0:128] f=[128:256] i=[256:384]
                        # a=[384:512]. PSUM holds SW*z.
                        obw = slice(ob * NW, (ob + 1) * NW)
                        p1, p2 = p1s[m], p2s[m]
                        zr = epp.tile([128, NW], F32, tag="zra", name="zra")
                        nc.vector.tensor_sub(zr[:], p1[:], p2[:])
                        nc.gpsimd.tensor_add(zr[:], zr[:], bias_r[:, obw])
                        gr = epp.tile([128, NW], F32, tag=f"gr_{m}",
                                      name=f"gr_{m}", bufs=1)
                        nc.scalar.activation(gr[:, 0:384], zr[:, 0:384],
                                             AFT.Sigmoid, scale=1.0 / SP)
                        nc.scalar.activation(gr[:, 384:512], zr[:, 384:512],
                                             AFT.Tanh, scale=1.0 / SP)
                        # q = p1 + p2 - bias_i (scaled): phase B does
                        # zi = P3 - q in one DVE op off the bias path
                        q = epp.tile([128, NW], F32, tag=f"q_{m}",
                                     name=f"q_{m}", bufs=1)
                        nc.vector.tensor_add(q[:], p1[:], p2[:])
                        nc.gpsimd.tensor_sub(q[:], q[:], bias_i[:, obw])
                        cr = ctile[m][:, ocols]
                        ci = ctile[m][:, oicols]
                        ir_ = gr[:, 0:128]
                        fr = gr[:, 128:256]
                        orr = gr[:, 256:384]
                        ar = gr[:, 384:512]
                        u1 = prodp.tile([128, 128], F32, tag=f"u1_{m}",
                                        name=f"u1_{m}", bufs=1)
                        u4 = prodp.tile([128, 128], F32, tag=f"u4_{m}",
                                        name=f"u4_{m}", bufs=1)
                        v1 = prodp.tile([128, 128], F32, tag=f"v1_{m}",
                                        name=f"v1_{m}", bufs=1)
                        nc.vector.tensor_mul(u1[:], cr, fr)
                        nc.vector.tensor_mul(u4[:], ci, fr)
                        nc.vector.tensor_mul(v1[:], ar, ir_)
                        pa[m] = (gr, q, u1, u4, v1)
                    else:
                        # ---- epilogue phase B for (ob, m): ps holds P3 ----
                        gr, q, u1, u4, v1 = pa[m]
                        zi = epp.tile([128, NW], F32, tag="zi", name="zi")
                        # halves: the sigmoid can start after the first
                        # half-subtract instead of the full-width op
                        nc.vector.tensor_sub(zi[:, 0:256], ps[:, 0:256],
                                             q[:, 0:256])
                        nc.vector.tensor_sub(zi[:, 256:512], ps[:, 256:512],
                                             q[:, 256:512])
                        gi = epp.tile([128, NW], F32, tag="gi", name="gi")
                        nc.scalar.activation(gi[:, 0:256], zi[:, 0:256],
                                             AFT.Sigmoid, scale=1.0 / SP)
                        nc.scalar.activation(gi[:, 256:384], zi[:, 256:384],
                                             AFT.Sigmoid, scale=1.0 / SP)
                        nc.scalar.activation(gi[:, 384:512], zi[:, 384:512],
                                             AFT.Tanh, scale=1.0 / SP)
                        cr = ctile[m][:, ocols]
                        ci = ctile[m][:, oicols]
                        ii_ = gi[:, 0:128]
                        fi = gi[:, 128:256]
                        oi = gi[:, 256:384]
                        ai = gi[:, 384:512]
                        ir_ = gr[:, 0:128]
                        orr = gr[:, 256:384]
                        ar = gr[:, 384:512]
                        u2 = prodp.tile([128, 128], F32, tag="u2", name="u2")
                        u3 = prodp.tile([128, 128], F32, tag="u3", name="u3")
                        v2 = prodp.tile([128, 128], F32, tag="v2", name="v2")
                        v3 = prodp.tile([128, 128], F32, tag="v3", name="v3")
                        v4 = prodp.tile([128, 128], F32, tag="v4", name="v4")
                        nc.vector.tensor_mul(u2[:], ci, fi)
                        nc.vector.tensor_mul(u3[:], cr, fi)
                        nc.gpsimd.tensor_mul(v2[:], ai, ii_)
                        nc.gpsimd.tensor_mul(v3[:], ar, ii_)
                        nc.vector.tensor_mul(v4[:], ai, ir_)
                        cfr = prodp.tile([128, 128], F32, tag="cfr", name="cfr")
                        cfi = prodp.tile([128, 128], F32, tag="cfi", name="cfi")
                        air = prodp.tile([128, 128], F32, tag="air", name="air")
                        aii = prodp.tile([128, 128], F32, tag="aii", name="aii")
                        nc.vector.tensor_sub(cfr[:], u1[:], u2[:])
                        nc.vector.tensor_add(cfi[:], u3[:], u4[:])
                        nc.gpsimd.tensor_sub(air[:], v1[:], v2[:])
                        nc.gpsimd.tensor_add(aii[:], v3[:], v4[:])
                        # c_t in bf16 (output dtype): tanh reads bf16 fine
                        ctr = prodp.tile([128, 128], BF16, tag="ctr", name="ctr")
                        cti = prodp.tile([128, 128], BF16, tag="cti", name="cti")
                        nc.vector.tensor_add(ctr[:], cfr[:], air[:])
                        nc.vector.tensor_add(cti[:], cfi[:], aii[:])
                        tr = prodp.tile([128, 128], F32, tag="tr", name="tr")
                        ti = prodp.tile([128, 128], F32, tag="ti", name="ti")
                        nc.scalar.activation(tr[:], ctr[:], AFT.Tanh)
                        nc.scalar.activation(ti[:], cti[:], AFT.Tanh)
                        htr = prodp.tile([128, 128], BF16, tag="htr", name="htr")
                        hti = prodp.tile([128, 128], BF16, tag="hti", name="hti")
                        w1 = prodp.tile([128, 128], F32, tag="w1", name="w1")
                        w2 = prodp.tile([128, 128], F32, tag="w2", name="w2")
                        w3 = prodp.tile([128, 128], F32, tag="w3", name="w3")
                        w4 = prodp.tile([128, 128], F32, tag="w4", name="w4")
                        # real half on DVE, imag half on GPSIMD in parallel
                        nc.vector.tensor_mul(w1[:], orr, tr[:])
                        nc.vector.tensor_mul(w2[:], oi, ti[:])
                        nc.vector.tensor_sub(htr[:], w1[:], w2[:])
                        nc.gpsimd.tensor_mul(w3[:], orr, ti[:])
                        nc.gpsimd.tensor_mul(w4[:], oi, tr[:])
                        nc.gpsimd.tensor_add(hti[:], w3[:], w4[:])
                        nc.sync.dma_start(out=h_out[rows, ocols], in_=htr[:])
                        nc.scalar.dma_start(out=h_out[rows, oicols], in_=hti[:])
                        nc.scalar.dma_start(out=c_out[rows, ocols], in_=ctr[:])
                        nc.sync.dma_start(out=c_out[rows, oicols], in_=cti[:])

            # PE stream order: X1 hi/lo loads, then the first matmul block
            # (only needs X1), then X2/X3 loads while that block runs.
            d1 = emit_xloads(xh[0], xht[0])
            if xlt[0] is not None:
                d1 += emit_xloads(xl[0], xlt[0])
            emit_mat(0, 0)
            d2 = []
            for m in (1, 2):
                d2 += emit_xloads(xh[m], xht[m])
                if xlt[m] is not None:
                    d2 += emit_xloads(xl[m], xlt[m])
            # host-broadcast bias tiles ride the SWDGE ring after all
            # x/h loads (first use is the phase-A gpsimd adds)
            for _bt, _bi in ((bias_r, 0), (bias_i, 1)):
                d = nc.gpsimd.dma_start(out=_bt[:], in_=bbc[_bi])
                add_dep_helper(d.ins, d2[-1].ins, sync=False,
                               reason="bias loads after x loads")
            # c tiles ride the SWDGE ring after all x loads (first use of
            # c is the phase-A products)
            for m in range(MT):
                t = cresp.tile([128, 2 * H], BF16, tag=f"c_m{m}", name=f"c_m{m}")
                d = nc.gpsimd.dma_start(out=t[:], in_=cx[m * 128 : (m + 1) * 128, :])
                add_dep_helper(d.ins, d2[-1].ins, sync=False,
                               reason="c loads after x loads")
                ctile.append(t)
            emit_mat(0, 1)
            emit_mat(0, 2)
            for ob in range(1, OB):
                for mat in range(3):
                    emit_mat(ob, mat)
    return nc


_NC_CACHE = None


def _get_program():
    global _NC_CACHE
    if _NC_CACHE is None:
        nc = _build_program()
        fixed = _split_multiwait_json(nc.to_json_bytes())
        nc.to_json_bytes = lambda: fixed
        _NC_CACHE = nc
    return _NC_CACHE


F8NP = ml_dtypes.float8_e4m3


def _split8(a):
    a = a * SX
    ah = a.astype(F8NP)
    al = (a - ah.astype(np.float32)).astype(F8NP)
    return ah, al


def _pack_weights(Uw_r, Uw_i, Ub_r, Ub_i, Ww_r, Ww_i, Wb_r, Wb_i):
    GORD = [1, 0, 3, 2]  # column blocks [i, f, o, a]: sigmoid trio is
    # [0:384] (one act call), tanh at [384:512]; correction suffixes cover
    # gates in sensitivity order a > o > f > i.

    def interleave_cols(Wg):  # [2048, G, H] -> [2048, GH]
        return (
            Wg.reshape(K, G, OB, 128)[:, GORD]
            .transpose(0, 2, 1, 3)
            .reshape(K, G * H)
        )

    Wr = np.concatenate(
        [np.transpose(Uw_r, (2, 0, 1)), np.transpose(Ww_r, (2, 0, 1))], axis=0
    )
    Wi = np.concatenate(
        [np.transpose(Uw_i, (2, 0, 1)), np.transpose(Ww_i, (2, 0, 1))], axis=0
    )
    W1 = interleave_cols(Wr) * SW
    W2 = interleave_cols(Wi) * SW
    W3 = W1 + W2
    Wall = np.stack([W1, W2, W3])  # [3, 2048, 4096] f32, pre-scaled

    def slabify(Wm, cols):  # [2048, ncols] -> [ob, 128, KT, ncols_per_ob]
        ncpo = cols
        return (
            Wm.reshape(KT, 128, OB, ncpo)
            .transpose(2, 1, 0, 3)
        )

    whs = []
    wls = []
    for m in range(3):
        Wh8 = Wall[m].astype(F8NP)
        Wl8 = (Wall[m] - Wh8.astype(np.float32)).astype(F8NP)
        whs.append(
            Wh8.reshape(KT, 128, OB, NW).transpose(2, 1, 0, 3)
        )
        wc0 = CORR_CFG[m][1]
        if wc0 < NW:
            wlm = (
                Wl8.reshape(KT, 128, OB, NW)
                .transpose(2, 1, 0, 3)[:, :, :, wc0:]
            )
            wls.append(np.ascontiguousarray(wlm))
        else:
            wls.append(None)
    wh = np.ascontiguousarray(np.stack(whs))  # [3, OB, 128, KT, NW]

    def interleave_bias(b):  # [G, H] -> [GH] interleaved, pre-scaled
        return b.reshape(G, OB, 128)[GORD].transpose(1, 0, 2).reshape(G * H)

    br = interleave_bias((Ub_r + Wb_r) * SP)
    bi = interleave_bias((Ub_i + Wb_i) * SP)
    bbc = np.ascontiguousarray(np.broadcast_to(
        np.stack([br, bi])[:, None, :], (2, 128, G * H)
    ).astype(ml_dtypes.bfloat16))
    return wh, wls, bbc


def kernel(input, h_x, c_x, Uw_r, Uw_i, Ub_r, Ub_i, Ww_r, Ww_i, Wb_r, Wb_i,
           _trace=False):
    input = np.asarray(input, dtype=np.float32)
    h_x = np.asarray(h_x, dtype=np.float32)
    c_x = np.asarray(c_x, dtype=np.float32)
    wh, wls, bpk = _pack_weights(
        np.asarray(Uw_r, np.float32), np.asarray(Uw_i, np.float32),
        np.asarray(Ub_r, np.float32), np.asarray(Ub_i, np.float32),
        np.asarray(Ww_r, np.float32), np.asarray(Ww_i, np.float32),
        np.asarray(Wb_r, np.float32), np.asarray(Wb_i, np.float32),
    )

    X1 = np.concatenate([input[:, :IN], h_x[:, :H]], axis=1)
    X2 = np.concatenate([input[:, IN:], h_x[:, H:]], axis=1)
    X3 = X1 + X2
    xparts = [_split8(X) for X in (X1, X2, X3)]

    in_maps = []
    for c in range(NCORES):
        rows = slice(c * BL, (c + 1) * BL)
        im = {
            "cx": np.ascontiguousarray(c_x[rows].astype(ml_dtypes.bfloat16)),
            "wh": wh,
            "bbc": bpk,
        }
        for m in range(3):
            xhm, xlm = xparts[m]
            im[f"x{m}h"] = np.ascontiguousarray(
                xhm[rows].T.reshape(KT, 128, BL)
            )
            if CORR_CFG[m][0] < NW:
                im[f"x{m}l"] = np.ascontiguousarray(
                    xlm[rows].T.reshape(KT, 128, BL)
                )
            if wls[m] is not None:
                im[f"wl{m}"] = wls[m]
        in_maps.append(im)

    nc = _get_program()
    res = run_bass_kernel_spmd(
        nc, in_maps, core_ids=list(range(NCORES)), trace=_trace
    )
    h_t = np.concatenate(
        [res.results[i]["h_out"].astype(np.float32) for i in range(NCORES)],
        axis=0,
    )
    c_t = np.concatenate(
        [res.results[i]["c_out"].astype(np.float32) for i in range(NCORES)],
        axis=0,
    )
    if _trace:
        kernel._last_results = res
    return h_t, c_t


# revision 13
# speedup vs baseline: 1.5153x; 1.0326x over previous
"""Complex LSTM cell (CLSTMCell) Trainium2 kernel — fp8 DoubleRow edition.

Full inputs in, full outputs out. Data-parallel over batch: B=4096 rows
sharded 512/core across 8 NeuronCores; the weight matrices are replicated
(host pre-packed into a matmul-friendly fp8 layout).

Math: with X1=[xr|hr], X2=[xi|hi] ([B,2048]) and W1=[Ur;Wr], W2=[Ui;Wi]
([2048,4096]), the complex gate projection is computed via Karatsuba:
  P1 = X1@W1, P2 = X2@W2, P3 = (X1+X2)@(W1+W2)
  Zr = P1 - P2 (+ br),  Zi = P3 - P1 - P2 (+ bi)
i.e. 3 real matmuls instead of 4 (25% FLOP cut).

Matmuls run in fp8-e4m3 with MatmulPerfMode.DoubleRow (two k-subtiles per
instruction at 0.5 cycles/row = 4x bf16 PE throughput). fp8's 3-bit
mantissa alone is too coarse (rel err ~5e-2 > 2e-2 gate), so each operand
is split hi/lo: X*4 = Xh + Xl, W*1024 = Wh + Wl (all four parts fp8;
the 4096 product scale folds out via the activation `scale` arg). The
product is corrected per gate-column block:
  P = Xh@Wh [+ Xl@Wh (X-corr)] [+ Xh@Wl (W-corr)]
Correction column sets are per-mat suffixes of the gate order [i,f,o,a]
(CORR_CFG below), chosen by offline error search: less-sensitive gates
skip corrections so only the columns that matter pay the extra DR passes.

Weight columns are interleaved as c = oblk*512 + gate*128 + (o % 128)
with gate order [i,f,o,a], so each N=512 matmul block contains all 4
gates for one 128-wide o slice, letting the cell update complete
per-block with no cross-block buffering.
"""

import sys

for _p in ("/opt/trn_rl_repo",):
    if _p not in sys.path:
        sys.path.insert(0, _p)

import numpy as np
import ml_dtypes

import concourse.bass as bass
import concourse.mybir as mybir
from concourse.bass_utils import run_bass_kernel_spmd
from concourse.tile import TileContext, add_dep_helper

F32 = mybir.dt.float32
BF16 = mybir.dt.bfloat16
F8 = mybir.dt.float8e4
AFT = mybir.ActivationFunctionType
DR = mybir.MatmulPerfMode.DoubleRow

B = 4096
IN = 1024
H = 1024
G = 4
NCORES = 8
BL = B // NCORES          # 512 batch rows per core
MT = BL // 128            # 4 m-tiles per core
K = 2 * IN                # 2048 contraction dim (x|h concat)
KT = K // 128             # 16 k-tiles
OB = H // 128             # 8 o-blocks
NW = G * 128              # 512 matmul N (all gates for one o-block)
SX = 4.0                  # x-side pre-scale
SW = 1024.0               # weight pre-scale
SP = SX * SW              # product scale, folded out via activation scale
# Scales keep all four fp8 operand classes (Xh, Xl, Wh, Wl) out of e4m3's
# subnormal range (hi parts sigma ~4 / ~22; residuals sigma ~0.07 / ~0.4,
# vs tiny=0.0156), so correctness survives even if the PE flushes fp8
# subnormals (the interpreter doesn't, hardware behavior unverified).

# Gate order within each 512-wide o-block: [i, f, o, a] (measured output
# sensitivity to z-error: a > o > f > i).
# Column slices: i=[0:128] f=[128:256] o=[256:384] a=[384:512].
# Correction sets are suffixes [start:512]; per mat (P1, P2, P3):
#   (xcorr_start, wcorr_start), 512 = no correction of that side.
# Offline greedy search: skipping i-gate corrections on P1/P2 and the
# P2 f-gate W-corr lands at ~1.4e-2 hw rel err vs the 2e-2 gate.
CORR_CFG = [(128, 128), (128, 256), (0, 0)]


def _split_multiwait_json(raw: bytes) -> bytes:
    """The walrus build in this container accepts at most one sem wait
    per instruction; Tile's scheduler packs several. Split the extras
    into preceding wait-only EventSemaphore instructions on the same
    engine (same semantics: the sequencer blocks on each in order)."""
    import orjson

    m = orjson.loads(raw)
    ctr = 0
    for fn in m["functions"]:
        for bb in fn["blocks"]:
            out = []
            for ins in bb["instructions"]:
                si = ins.get("sync_info")
                waits = si.get("on_wait") if si else None
                if waits and len(waits) > 1:
                    for w in waits[:-1]:
                        ctr += 1
                        nop = {
                            "engine": ins["engine"],
                            "ins": [],
                            "outs": [],
                            "name": f"{ins['name']}_sw{ctr}",
                            "opcode": "EventSemaphore",
                            "sync_info": {"on_update": [], "on_wait": [w]},
                        }
                        if "debug" in ins:
                            nop["debug"] = ins["debug"]
                        out.append(nop)
                    si["on_wait"] = [waits[-1]]
                out.append(ins)
            bb["instructions"] = out
    return orjson.dumps(m)


def _build_program():
    nc = bass.Bass()

    # x-side: hi/lo fp8 for X1, X2, X3, pre-transposed on host: [KT,128,BL]
    xh = [nc.dram_tensor(f"x{m}h", [KT, 128, BL], F8, kind="ExternalInput")
          for m in range(3)]
    xl = [nc.dram_tensor(f"x{m}l", [KT, 128, BL], F8, kind="ExternalInput")
          for m in range(3)]
    cx = nc.dram_tensor("cx", [BL, 2 * H], BF16, kind="ExternalInput")
    # hi slabs: [3, OB, 128, KT, NW]; lo slabs only for corrected columns
    wh = nc.dram_tensor("wh", [3, OB, 128, KT, NW], F8, kind="ExternalInput")
    wl = [
        nc.dram_tensor(f"wl{m}", [OB, 128, KT, NW - CORR_CFG[m][1]], F8,
                       kind="ExternalInput")
        if CORR_CFG[m][1] < NW else None
        for m in range(3)
    ]
    bbc = nc.dram_tensor("bbc", [2, 128, G * H], BF16, kind="ExternalInput")
    h_out = nc.dram_tensor("h_out", [BL, 2 * H], BF16, kind="ExternalOutput")
    c_out = nc.dram_tensor("c_out", [BL, 2 * H], BF16, kind="ExternalOutput")

    with TileContext(nc) as tc:
        with (
            tc.tile_pool(name="const", bufs=1) as constp,
            tc.tile_pool(name="cres", bufs=1) as cresp,
            tc.tile_pool(name="xt", bufs=1) as xtp,
            tc.tile_pool(name="w", bufs=3) as wp,
            tc.tile_pool(name="wlp", bufs=3) as wlp,
            tc.tile_pool(name="pp", bufs=1) as pp,
            tc.tile_pool(name="ep", bufs=2) as epp,
            tc.tile_pool(name="prod", bufs=2) as prodp,
            tc.tile_pool(name="ps_mm", bufs=6, space="PSUM") as psmm,
        ):
            bias_r = constp.tile([128, G * H], BF16, tag="bias_r", name="bias_r")
            bias_i = constp.tile([128, G * H], BF16, tag="bias_i", name="bias_i")

            ctile = []

            # X tiles: one [128, KT, BL] tile per tensor; DR slices
            # [:, 2k:2k+2, m*128:(m+1)*128] need k-pairs adjacent in dim1.
            xht = [xtp.tile([128, KT, BL], F8, tag=f"xh{m}", name=f"xh{m}")
                   for m in range(3)]
            xlt = [
                xtp.tile([128, KT, BL], F8, tag=f"xl{m}", name=f"xl{m}")
                if CORR_CFG[m][0] < NW else None
                for m in range(3)
            ]

            def emit_xloads(src, dst, quarters=4, first_pair_fast=False):
                # dram [KT,128,BL] -> sbuf [128, KT, BL] in `quarters` DMAs;
                # first_pair_fast peels k-tiles 0-1 onto the idle DVE HWDGE
                # ring so the very first matmul unblocks sooner (ACT is idle
                # at t=0; DVE cannot issue DMAs in this build).
                dmas = []
                if first_pair_fast:
                    dmas.append(nc.scalar.dma_start(
                        out=dst[:, 0:2, :],
                        in_=src[0:2].rearrange("kt p b -> p kt b"),
                    ))
                    dmas.append(nc.scalar.dma_start(
                        out=dst[:, 2:4, :],
                        in_=src[2:4].rearrange("kt p b -> p kt b"),
                    ))
                    start = 1
                else:
                    start = 0
                kq = KT // quarters
                for q in range(start, quarters):
                    dmas.append(nc.gpsimd.dma_start(
                        out=dst[:, q * kq : (q + 1) * kq, :],
                        in_=src[q * kq : (q + 1) * kq].rearrange(
                            "kt p b -> p kt b"
                        ),
                    ))
                return dmas

            p1s_all = {ob: [None] * MT for ob in range(OB)}
            p2s_all = {ob: [None] * MT for ob in range(OB)}
            pa = [None] * MT

            def emit_mat(ob, mat, ms=None, wtiles=None):
                ocols = slice(ob * 128, (ob + 1) * 128)
                oicols = slice(H + ob * 128, H + (ob + 1) * 128)
                p1s = p1s_all[ob]
                p2s = p2s_all[ob]
                xc0, wc0 = CORR_CFG[mat]
                if ms is None:
                    ms = range(MT)
                if wtiles is not None:
                    w, wlt = wtiles
                    return emit_mat_body(ob, mat, ms, w, wlt, ocols, oicols,
                                         p1s, p2s, xc0, wc0)
                w = wp.tile([128, KT, NW], F8, tag="wslab", name="wslab")
                wsrc = wh[mat, ob].rearrange("p kt c -> p (kt c)")
                wdst = w.rearrange("p kt c -> p (kt c)")
                if ob == 0 and mat == 0:
                    # split the very first slab so matmuls start as
                    # chunks land
                    qr = KT * NW // 8
                    for _q in range(8):
                        nc.sync.dma_start(
                            out=wdst[:, _q * qr : (_q + 1) * qr],
                            in_=wsrc[:, _q * qr : (_q + 1) * qr],
                        )
                elif ob == 0 and mat == 1:
                    half = KT * NW // 2
                    nc.sync.dma_start(out=wdst[:, :half], in_=wsrc[:, :half])
                    nc.sync.dma_start(out=wdst[:, half:], in_=wsrc[:, half:])
                else:
                    nc.sync.dma_start(out=wdst[:], in_=wsrc)
                wlt = None
                if wc0 < NW:
                    ncw = NW - wc0
                    wlt = wlp.tile([128, KT, ncw], F8, tag="wlslab",
                                   name="wlslab")
                    nc.sync.dma_start(
                        out=wlt.rearrange("p kt c -> p (kt c)"),
                        in_=wl[mat][ob].rearrange("p kt c -> p (kt c)"),
                    )
                return emit_mat_body(ob, mat, ms, w, wlt, ocols, oicols,
                                     p1s, p2s, xc0, wc0)

            def emit_mat_body(ob, mat, ms, w, wlt, ocols, oicols,
                              p1s, p2s, xc0, wc0):
                for m in ms:
                    rows = slice(m * 128, (m + 1) * 128)
                    ps = psmm.tile([128, NW], F32, tag="mm", name="mm")
                    main_is_last = xc0 >= NW and wc0 >= NW
                    # main pass: Xh @ Wh, full width
                    for k in range(0, KT, 2):
                        nc.tensor.matmul(
                            ps[:],
                            lhsT=xht[mat][:, k : k + 2, rows],
                            rhs=w[:, k : k + 2, :],
                            start=(k == 0),
                            stop=(main_is_last and k == KT - 2),
                            perf_mode=DR,
                        )
                    # Correction passes, sliced so the sigmoid-gate
                    # columns [*:384] complete before the a-gate [384:512]
                    # ones: phase A/B readers of those columns unblock while
                    # the PE still runs a-gate corrections.
                    corr = []
                    if xc0 < NW:
                        corr.append(("x", xc0, NW))
                    if wc0 < NW:
                        corr.append(("w", wc0, NW))
                    for ci, (side, c0, c1) in enumerate(corr):
                        for k in range(0, KT, 2):
                            if side == "x":
                                lhsT = xlt[mat][:, k : k + 2, rows]
                                rhs = w[:, k : k + 2, c0:c1]
                            else:
                                lhsT = xht[mat][:, k : k + 2, rows]
                                rhs = wlt[:, k : k + 2, c0 - wc0 : c1 - wc0]
                            nc.tensor.matmul(
                                ps[:, c0:c1],
                                lhsT=lhsT,
                                rhs=rhs,
                                start=False,
                                stop=(ci == len(corr) - 1 and k == KT - 2),
                                perf_mode=DR,
                            )

                    if mat == 0:
                        p1s[m] = pp.tile([128, NW], F32, tag=f"p1_{m}",
                                         name=f"p1_{m}")
                        nc.scalar.copy(p1s[m][:], ps[:])
                    elif mat == 1:
                        p2s[m] = pp.tile([128, NW], F32, tag=f"p2_{m}",
                                         name=f"p2_{m}")
                        nc.scalar.copy(p2s[m][:], ps[:])
                        # ---- epilogue phase A: everything that only
                        # needs P1/P2 (not P3) — overlaps the P3 matmuls.
                        # Gate cols: o=[0:128] f=[128:256] i=[256:384]
                        # a=[384:512]. PSUM holds SW*z.
                        obw = slice(ob * NW, (ob + 1) * NW)
                        p1, p2 = p1s[m], p2s[m]
                        zr = epp.tile([128, NW], F32, tag="zra", name="zra")
                        nc.vector.tensor_sub(zr[:], p1[:], p2[:])
                        nc.gpsimd.tensor_add(zr[:], zr[:], bias_r[:, obw])
                        gr = epp.tile([128, NW], F32, tag=f"gr_{m}",
                                      name=f"gr_{m}", bufs=1)
                        nc.scalar.activation(gr[:, 0:384], zr[:, 0:384],
                                             AFT.Sigmoid, scale=1.0 / SP)
                        nc.scalar.activation(gr[:, 384:512], zr[:, 384:512],
                                             AFT.Tanh, scale=1.0 / SP)
                        # q = p1 + p2 - bias_i (scaled): phase B does
                        # zi = P3 - q in one DVE op off the bias path
                        q = epp.tile([128, NW], F32, tag=f"q_{m}",
                                     name=f"q_{m}", bufs=1)
                        nc.vector.tensor_add(q[:], p1[:], p2[:])
                        nc.gpsimd.tensor_sub(q[:], q[:], bias_i[:, obw])
                        cr = ctile[m][:, ocols]
                        ci = ctile[m][:, oicols]
                        ir_ = gr[:, 0:128]
                        fr = gr[:, 128:256]
                        orr = gr[:, 256:384]
                        ar = gr[:, 384:512]
                        u1 = prodp.tile([128, 128], F32, tag=f"u1_{m}",
                                        name=f"u1_{m}", bufs=1)
                        u4 = prodp.tile([128, 128], F32, tag=f"u4_{m}",
                                        name=f"u4_{m}", bufs=1)
                        v1 = prodp.tile([128, 128], F32, tag=f"v1_{m}",
                                        name=f"v1_{m}", bufs=1)
                        nc.vector.tensor_mul(u1[:], cr, fr)
                        nc.vector.tensor_mul(u4[:], ci, fr)
                        nc.vector.tensor_mul(v1[:], ar, ir_)
                        pa[m] = (gr, q, u1, u4, v1)
                    else:
                        # ---- epilogue phase B for (ob, m): ps holds P3 ----
                        gr, q, u1, u4, v1 = pa[m]
                        zi = epp.tile([128, NW], F32, tag="zi", name="zi")
                        # halves: the sigmoid can start after the first
                        # half-subtract instead of the full-width op
                        nc.vector.tensor_sub(zi[:, 0:384], ps[:, 0:384],
                                             q[:, 0:384])
                        nc.vector.tensor_sub(zi[:, 384:512], ps[:, 384:512],
                                             q[:, 384:512])
                        gi = epp.tile([128, NW], F32, tag="gi", name="gi")
                        nc.scalar.activation(gi[:, 0:384], zi[:, 0:384],
                                             AFT.Sigmoid, scale=1.0 / SP)
                        nc.scalar.activation(gi[:, 384:512], zi[:, 384:512],
                                             AFT.Tanh, scale=1.0 / SP)
                        cr = ctile[m][:, ocols]
                        ci = ctile[m][:, oicols]
                        ii_ = gi[:, 0:128]
                        fi = gi[:, 128:256]
                        oi = gi[:, 256:384]
                        ai = gi[:, 384:512]
                        ir_ = gr[:, 0:128]
                        orr = gr[:, 256:384]
                        ar = gr[:, 384:512]
                        u2 = prodp.tile([128, 128], F32, tag="u2", name="u2")
                        u3 = prodp.tile([128, 128], F32, tag="u3", name="u3")
                        v2 = prodp.tile([128, 128], F32, tag="v2", name="v2")
                        v3 = prodp.tile([128, 128], F32, tag="v3", name="v3")
                        v4 = prodp.tile([128, 128], F32, tag="v4", name="v4")
                        nc.vector.tensor_mul(u2[:], ci, fi)
                        nc.vector.tensor_mul(u3[:], cr, fi)
                        nc.gpsimd.tensor_mul(v2[:], ai, ii_)
                        nc.gpsimd.tensor_mul(v3[:], ar, ii_)
                        nc.vector.tensor_mul(v4[:], ai, ir_)
                        cfr = prodp.tile([128, 128], F32, tag="cfr", name="cfr")
                        cfi = prodp.tile([128, 128], F32, tag="cfi", name="cfi")
                        air = prodp.tile([128, 128], F32, tag="air", name="air")
                        aii = prodp.tile([128, 128], F32, tag="aii", name="aii")
                        nc.vector.tensor_sub(cfr[:], u1[:], u2[:])
                        nc.vector.tensor_add(cfi[:], u3[:], u4[:])
                        nc.gpsimd.tensor_sub(air[:], v1[:], v2[:])
                        nc.gpsimd.tensor_add(aii[:], v3[:], v4[:])
                        # c_t in bf16 (output dtype): tanh reads bf16 fine
                        ctr = prodp.tile([128, 128], BF16, tag="ctr", name="ctr")
                        cti = prodp.tile([128, 128], BF16, tag="cti", name="cti")
                        nc.vector.tensor_add(ctr[:], cfr[:], air[:])
                        nc.vector.tensor_add(cti[:], cfi[:], aii[:])
                        tr = prodp.tile([128, 128], F32, tag="tr", name="tr")
                        ti = prodp.tile([128, 128], F32, tag="ti", name="ti")
                        nc.scalar.activation(tr[:], ctr[:], AFT.Tanh)
                        nc.scalar.activation(ti[:], cti[:], AFT.Tanh)
                        htr = prodp.tile([128, 128], BF16, tag="htr", name="htr")
                        hti = prodp.tile([128, 128], BF16, tag="hti", name="hti")
                        w1 = prodp.tile([128, 128], F32, tag="w1", name="w1")
                        w2 = prodp.tile([128, 128], F32, tag="w2", name="w2")
                        w3 = prodp.tile([128, 128], F32, tag="w3", name="w3")
                        w4 = prodp.tile([128, 128], F32, tag="w4", name="w4")
                        # real half on DVE, imag half on GPSIMD in parallel
                        nc.vector.tensor_mul(w1[:], orr, tr[:])
                        nc.vector.tensor_mul(w2[:], oi, ti[:])
                        nc.vector.tensor_sub(htr[:], w1[:], w2[:])
                        nc.gpsimd.tensor_mul(w3[:], orr, ti[:])
                        nc.gpsimd.tensor_mul(w4[:], oi, tr[:])
                        nc.gpsimd.tensor_add(hti[:], w3[:], w4[:])
                        nc.sync.dma_start(out=h_out[rows, ocols], in_=htr[:])
                        nc.gpsimd.dma_start(out=h_out[rows, oicols], in_=hti[:])
                        nc.gpsimd.dma_start(out=c_out[rows, ocols], in_=ctr[:])
                        nc.sync.dma_start(out=c_out[rows, oicols], in_=cti[:])

            # PE stream order: X1 hi/lo loads, then the first matmul block
            # (only needs X1), then X2/X3 loads while that block runs.
            d1 = emit_xloads(xh[0], xht[0], first_pair_fast=True)
            if xlt[0] is not None:
                d1 += emit_xloads(xl[0], xlt[0])
            emit_mat(0, 0)
            d2 = []
            for m in (1, 2):
                d2 += emit_xloads(xh[m], xht[m])
                if xlt[m] is not None:
                    d2 += emit_xloads(xl[m], xlt[m])
            # host-broadcast bias tiles ride the SWDGE ring after all
            # x/h loads (first use is the phase-A gpsimd adds)
            for _bt, _bi in ((bias_r, 0), (bias_i, 1)):
                d = nc.gpsimd.dma_start(out=_bt[:], in_=bbc[_bi])
                add_dep_helper(d.ins, d2[-1].ins, sync=False,
                               reason="bias loads after x loads")
            # c tiles ride the SWDGE ring after all x loads (first use of
            # c is the phase-A products)
            for m in range(MT):
                t = cresp.tile([128, 2 * H], BF16, tag=f"c_m{m}", name=f"c_m{m}")
                d = nc.gpsimd.dma_start(out=t[:], in_=cx[m * 128 : (m + 1) * 128, :])
                add_dep_helper(d.ins, d2[-1].ins, sync=False,
                               reason="c loads after x loads")
                ctile.append(t)
            emit_mat(0, 1)
            emit_mat(0, 2)
            for ob in range(1, OB):
                for mat in range(3):
                    if ob == OB - 1 and mat == 2:
                        # split the final P3 so only two phase-B chains
                        # drain after the last matmul
                        wt = (wp.tile([128, KT, NW], F8, tag="wslab",
                                      name="wslab"),
                              wlp.tile([128, KT, NW - CORR_CFG[2][1]], F8,
                                       tag="wlslab", name="wlslab")
                              if CORR_CFG[2][1] < NW else None)
                        nc.sync.dma_start(
                            out=wt[0].rearrange("p kt c -> p (kt c)"),
                            in_=wh[2, ob].rearrange("p kt c -> p (kt c)"),
                        )
                        if wt[1] is not None:
                            nc.sync.dma_start(
                                out=wt[1].rearrange("p kt c -> p (kt c)"),
                                in_=wl[2][ob].rearrange("p kt c -> p (kt c)"),
                            )
                        emit_mat(ob, mat, ms=range(0, 2), wtiles=wt)
                        emit_mat(ob, mat, ms=range(2, MT), wtiles=wt)
                    else:
                        emit_mat(ob, mat)
    return nc


_NC_CACHE = None


def _get_program():
    global _NC_CACHE
    if _NC_CACHE is None:
        nc = _build_program()
        fixed = _split_multiwait_json(nc.to_json_bytes())
        nc.to_json_bytes = lambda: fixed
        _NC_CACHE = nc
    return _NC_CACHE


F8NP = ml_dtypes.float8_e4m3


def _split8(a):
    a = a * SX
    ah = a.astype(F8NP)
    al = (a - ah.astype(np.float32)).astype(F8NP)
    return ah, al


def _pack_weights(Uw_r, Uw_i, Ub_r, Ub_i, Ww_r, Ww_i, Wb_r, Wb_i):
    GORD = [1, 0, 3, 2]  # column blocks [i, f, o, a]: sigmoid trio is
    # [0:384] (one act call), tanh at [384:512]; correction suffixes cover
    # gates in sensitivity order a > o > f > i.

    def interleave_cols(Wg):  # [2048, G, H] -> [2048, GH]
        return (
            Wg.reshape(K, G, OB, 128)[:, GORD]
            .transpose(0, 2, 1, 3)
            .reshape(K, G * H)
        )

    Wr = np.concatenate(
        [np.transpose(Uw_r, (2, 0, 1)), np.transpose(Ww_r, (2, 0, 1))], axis=0
    )
    Wi = np.concatenate(
        [np.transpose(Uw_i, (2, 0, 1)), np.transpose(Ww_i, (2, 0, 1))], axis=0
    )
    W1 = interleave_cols(Wr) * SW
    W2 = interleave_cols(Wi) * SW
    W3 = W1 + W2
    Wall = np.stack([W1, W2, W3])  # [3, 2048, 4096] f32, pre-scaled

    def slabify(Wm, cols):  # [2048, ncols] -> [ob, 128, KT, ncols_per_ob]
        ncpo = cols
        return (
            Wm.reshape(KT, 128, OB, ncpo)
            .transpose(2, 1, 0, 3)
        )

    whs = []
    wls = []
    for m in range(3):
        Wh8 = Wall[m].astype(F8NP)
        Wl8 = (Wall[m] - Wh8.astype(np.float32)).astype(F8NP)
        whs.append(
            Wh8.reshape(KT, 128, OB, NW).transpose(2, 1, 0, 3)
        )
        wc0 = CORR_CFG[m][1]
        if wc0 < NW:
            wlm = (
                Wl8.reshape(KT, 128, OB, NW)
                .transpose(2, 1, 0, 3)[:, :, :, wc0:]
            )
            wls.append(np.ascontiguousarray(wlm))
        else:
            wls.append(None)
    wh = np.ascontiguousarray(np.stack(whs))  # [3, OB, 128, KT, NW]

    def interleave_bias(b):  # [G, H] -> [GH] interleaved, pre-scaled
        return b.reshape(G, OB, 128)[GORD].transpose(1, 0, 2).reshape(G * H)

    br = interleave_bias((Ub_r + Wb_r) * SP)
    bi = interleave_bias((Ub_i + Wb_i) * SP)
    bbc = np.ascontiguousarray(np.broadcast_to(
        np.stack([br, bi])[:, None, :], (2, 128, G * H)
    ).astype(ml_dtypes.bfloat16))
    return wh, wls, bbc


def kernel(input, h_x, c_x, Uw_r, Uw_i, Ub_r, Ub_i, Ww_r, Ww_i, Wb_r, Wb_i,
           _trace=False):
    input = np.asarray(input, dtype=np.float32)
    h_x = np.asarray(h_x, dtype=np.float32)
    c_x = np.asarray(c_x, dtype=np.float32)
    wh, wls, bpk = _pack_weights(
        np.asarray(Uw_r, np.float32), np.asarray(Uw_i, np.float32),
        np.asarray(Ub_r, np.float32), np.asarray(Ub_i, np.float32),
        np.asarray(Ww_r, np.float32), np.asarray(Ww_i, np.float32),
        np.asarray(Wb_r, np.float32), np.asarray(Wb_i, np.float32),
    )

    X1 = np.concatenate([input[:, :IN], h_x[:, :H]], axis=1)
    X2 = np.concatenate([input[:, IN:], h_x[:, H:]], axis=1)
    X3 = X1 + X2
    xparts = [_split8(X) for X in (X1, X2, X3)]

    in_maps = []
    for c in range(NCORES):
        rows = slice(c * BL, (c + 1) * BL)
        im = {
            "cx": np.ascontiguousarray(c_x[rows].astype(ml_dtypes.bfloat16)),
            "wh": wh,
            "bbc": bpk,
        }
        for m in range(3):
            xhm, xlm = xparts[m]
            im[f"x{m}h"] = np.ascontiguousarray(
                xhm[rows].T.reshape(KT, 128, BL)
            )
            if CORR_CFG[m][0] < NW:
                im[f"x{m}l"] = np.ascontiguousarray(
                    xlm[rows].T.reshape(KT, 128, BL)
                )
            if wls[m] is not None:
                im[f"wl{m}"] = wls[m]
        in_maps.append(im)

    nc = _get_program()
    res = run_bass_kernel_spmd(
        nc, in_maps, core_ids=list(range(NCORES)), trace=_trace
    )
    h_t = np.concatenate(
        [res.results[i]["h_out"].astype(np.float32) for i in range(NCORES)],
        axis=0,
    )
    c_t = np.concatenate(
        [res.results[i]["c_out"].astype(np.float32) for i in range(NCORES)],
        axis=0,
    )
    if _trace:
        kernel._last_results = res
    return h_t, c_t


# revision 15
# speedup vs baseline: 1.5632x; 1.0316x over previous
"""Complex LSTM cell (CLSTMCell) Trainium2 kernel — fp8 DoubleRow edition.

Full inputs in, full outputs out. Data-parallel over batch: B=4096 rows
sharded 512/core across 8 NeuronCores; the weight matrices are replicated
(host pre-packed into a matmul-friendly fp8 layout).

Math: with X1=[xr|hr], X2=[xi|hi] ([B,2048]) and W1=[Ur;Wr], W2=[Ui;Wi]
([2048,4096]), the complex gate projection is computed via Karatsuba:
  P1 = X1@W1, P2 = X2@W2, P3 = (X1+X2)@(W1+W2)
  Zr = P1 - P2 (+ br),  Zi = P3 - P1 - P2 (+ bi)
i.e. 3 real matmuls instead of 4 (25% FLOP cut).

Matmuls run in fp8-e4m3 with MatmulPerfMode.DoubleRow (two k-subtiles per
instruction at 0.5 cycles/row = 4x bf16 PE throughput). fp8's 3-bit
mantissa alone is too coarse (rel err ~5e-2 > 2e-2 gate), so each operand
is split hi/lo: X*4 = Xh + Xl, W*1024 = Wh + Wl (all four parts fp8;
the 4096 product scale folds out via the activation `scale` arg). The
product is corrected per gate-column block:
  P = Xh@Wh [+ Xl@Wh (X-corr)] [+ Xh@Wl (W-corr)]
Correction column sets are per-mat suffixes of the gate order [i,f,o,a]
(CORR_CFG below), chosen by offline error search: less-sensitive gates
skip corrections so only the columns that matter pay the extra DR passes.

Weight columns are interleaved as c = oblk*512 + gate*128 + (o % 128)
with gate order [i,f,o,a], so each N=512 matmul block contains all 4
gates for one 128-wide o slice, letting the cell update complete
per-block with no cross-block buffering.
"""

import sys

for _p in ("/opt/trn_rl_repo",):
    if _p not in sys.path:
        sys.path.insert(0, _p)

import numpy as np
import ml_dtypes

import concourse.bass as bass
import concourse.mybir as mybir
from concourse.bass_utils import run_bass_kernel_spmd
from concourse.tile import TileContext, add_dep_helper

F32 = mybir.dt.float32
BF16 = mybir.dt.bfloat16
F8 = mybir.dt.float8e4
AFT = mybir.ActivationFunctionType
DR = mybir.MatmulPerfMode.DoubleRow

B = 4096
IN = 1024
H = 1024
G = 4
NCORES = 8
BL = B // NCORES          # 512 batch rows per core
MT = BL // 128            # 4 m-tiles per core
K = 2 * IN                # 2048 contraction dim (x|h concat)
KT = K // 128             # 16 k-tiles
OB = H // 128             # 8 o-blocks
NW = G * 128              # 512 matmul N (all gates for one o-block)
SX = 4.0                  # x-side pre-scale
SW = 1024.0               # weight pre-scale
SP = SX * SW              # product scale, folded out via activation scale
# Scales keep all four fp8 operand classes (Xh, Xl, Wh, Wl) out of e4m3's
# subnormal range (hi parts sigma ~4 / ~22; residuals sigma ~0.07 / ~0.4,
# vs tiny=0.0156), so correctness survives even if the PE flushes fp8
# subnormals (the interpreter doesn't, hardware behavior unverified).

# Gate order within each 512-wide o-block: [i, f, o, a] (measured output
# sensitivity to z-error: a > o > f > i).
# Column slices: i=[0:128] f=[128:256] o=[256:384] a=[384:512].
# Correction sets are suffixes [start:512]; per mat (P1, P2, P3):
#   (xcorr_start, wcorr_start), 512 = no correction of that side.
# Offline greedy search: skipping i-gate corrections on P1/P2 and the
# f-gate W-corrs lands at ~1.5e-2 hw rel err vs the 2e-2 gate.
CORR_CFG = [(128, 256), (128, 256), (0, 0)]


def _split_multiwait_json(raw: bytes) -> bytes:
    """The walrus build in this container accepts at most one sem wait
    per instruction; Tile's scheduler packs several. Split the extras
    into preceding wait-only EventSemaphore instructions on the same
    engine (same semantics: the sequencer blocks on each in order)."""
    import orjson

    m = orjson.loads(raw)
    ctr = 0
    for fn in m["functions"]:
        for bb in fn["blocks"]:
            out = []
            for ins in bb["instructions"]:
                si = ins.get("sync_info")
                waits = si.get("on_wait") if si else None
                if waits and len(waits) > 1:
                    for w in waits[:-1]:
                        ctr += 1
                        nop = {
                            "engine": ins["engine"],
                            "ins": [],
                            "outs": [],
                            "name": f"{ins['name']}_sw{ctr}",
                            "opcode": "EventSemaphore",
                            "sync_info": {"on_update": [], "on_wait": [w]},
                        }
                        if "debug" in ins:
                            nop["debug"] = ins["debug"]
                        out.append(nop)
                    si["on_wait"] = [waits[-1]]
                out.append(ins)
            bb["instructions"] = out
    return orjson.dumps(m)


def _build_program():
    nc = bass.Bass()

    # x-side: hi/lo fp8 for X1, X2, X3, pre-transposed on host: [KT,128,BL]
    xh = [nc.dram_tensor(f"x{m}h", [KT, 128, BL], F8, kind="ExternalInput")
          for m in range(3)]
    xl = [nc.dram_tensor(f"x{m}l", [KT, 128, BL], F8, kind="ExternalInput")
          for m in range(3)]
    cx = nc.dram_tensor("cx", [BL, 2 * H], BF16, kind="ExternalInput")
    # hi slabs: [3, OB, 128, KT, NW]; lo slabs only for corrected columns
    wh = nc.dram_tensor("wh", [3, OB, 128, KT, NW], F8, kind="ExternalInput")
    wl = [
        nc.dram_tensor(f"wl{m}", [OB, 128, KT, NW - CORR_CFG[m][1]], F8,
                       kind="ExternalInput")
        if CORR_CFG[m][1] < NW else None
        for m in range(3)
    ]
    bbc = nc.dram_tensor("bbc", [2, 128, G * H], BF16, kind="ExternalInput")
    h_out = nc.dram_tensor("h_out", [BL, 2 * H], BF16, kind="ExternalOutput")
    c_out = nc.dram_tensor("c_out", [BL, 2 * H], BF16, kind="ExternalOutput")

    with TileContext(nc) as tc:
        with (
            tc.tile_pool(name="const", bufs=1) as constp,
            tc.tile_pool(name="cres", bufs=1) as cresp,
            tc.tile_pool(name="xt", bufs=1) as xtp,
            tc.tile_pool(name="w", bufs=3) as wp,
            tc.tile_pool(name="wlp", bufs=3) as wlp,
            tc.tile_pool(name="pp", bufs=1) as pp,
            tc.tile_pool(name="ep", bufs=2) as epp,
            tc.tile_pool(name="prod", bufs=2) as prodp,
            tc.tile_pool(name="ps_mm", bufs=6, space="PSUM") as psmm,
        ):
            bias_r = constp.tile([128, G * H], BF16, tag="bias_r", name="bias_r")
            bias_i = constp.tile([128, G * H], BF16, tag="bias_i", name="bias_i")

            ctile = []

            # X tiles: one [128, KT, BL] tile per tensor; DR slices
            # [:, 2k:2k+2, m*128:(m+1)*128] need k-pairs adjacent in dim1.
            xht = [xtp.tile([128, KT, BL], F8, tag=f"xh{m}", name=f"xh{m}")
                   for m in range(3)]
            xlt = [
                xtp.tile([128, KT, BL], F8, tag=f"xl{m}", name=f"xl{m}")
                if CORR_CFG[m][0] < NW else None
                for m in range(3)
            ]

            def emit_xloads(src, dst, quarters=4, first_pair_fast=False):
                # dram [KT,128,BL] -> sbuf [128, KT, BL] in `quarters` DMAs;
                # first_pair_fast peels k-tiles 0-1 onto the idle DVE HWDGE
                # ring so the very first matmul unblocks sooner (ACT is idle
                # at t=0; DVE cannot issue DMAs in this build).
                dmas = []
                if first_pair_fast:
                    dmas.append(nc.scalar.dma_start(
                        out=dst[:, 0:2, :],
                        in_=src[0:2].rearrange("kt p b -> p kt b"),
                    ))
                    dmas.append(nc.scalar.dma_start(
                        out=dst[:, 2:4, :],
                        in_=src[2:4].rearrange("kt p b -> p kt b"),
                    ))
                    start = 1
                else:
                    start = 0
                kq = KT // quarters
                for q in range(start, quarters):
                    dmas.append(nc.gpsimd.dma_start(
                        out=dst[:, q * kq : (q + 1) * kq, :],
                        in_=src[q * kq : (q + 1) * kq].rearrange(
                            "kt p b -> p kt b"
                        ),
                    ))
                return dmas

            p1s_all = {ob: [None] * MT for ob in range(OB)}
            p2s_all = {ob: [None] * MT for ob in range(OB)}
            pa = [None] * MT

            def emit_mat(ob, mat, ms=None, wtiles=None, split_ag=False):
                ocols = slice(ob * 128, (ob + 1) * 128)
                oicols = slice(H + ob * 128, H + (ob + 1) * 128)
                p1s = p1s_all[ob]
                p2s = p2s_all[ob]
                xc0, wc0 = CORR_CFG[mat]
                if ms is None:
                    ms = range(MT)
                if wtiles is not None:
                    w, wlt = wtiles
                    return emit_mat_body(ob, mat, ms, w, wlt, ocols, oicols,
                                         p1s, p2s, xc0, wc0, split_ag)
                w = wp.tile([128, KT, NW], F8, tag="wslab", name="wslab")
                wsrc = wh[mat, ob].rearrange("p kt c -> p (kt c)")
                wdst = w.rearrange("p kt c -> p (kt c)")
                if ob == 0 and mat == 0:
                    # split the very first slab so matmuls start as
                    # chunks land
                    qr = KT * NW // 8
                    for _q in range(8):
                        nc.sync.dma_start(
                            out=wdst[:, _q * qr : (_q + 1) * qr],
                            in_=wsrc[:, _q * qr : (_q + 1) * qr],
                        )
                elif ob == 0 and mat == 1:
                    half = KT * NW // 2
                    nc.sync.dma_start(out=wdst[:, :half], in_=wsrc[:, :half])
                    nc.sync.dma_start(out=wdst[:, half:], in_=wsrc[:, half:])
                else:
                    nc.sync.dma_start(out=wdst[:], in_=wsrc)
                wlt = None
                if wc0 < NW:
                    ncw = NW - wc0
                    wlt = wlp.tile([128, KT, ncw], F8, tag="wlslab",
                                   name="wlslab")
                    nc.sync.dma_start(
                        out=wlt.rearrange("p kt c -> p (kt c)"),
                        in_=wl[mat][ob].rearrange("p kt c -> p (kt c)"),
                    )
                return emit_mat_body(ob, mat, ms, w, wlt, ocols, oicols,
                                     p1s, p2s, xc0, wc0, split_ag)

            def emit_mat_body(ob, mat, ms, w, wlt, ocols, oicols,
                              p1s, p2s, xc0, wc0, split_ag=False):
                def emit_group(ps, rows, c0, c1, mat):
                    # one accumulation group covering columns [c0:c1]
                    passes = [("m", max(xc0 * 0, c0), c1)]
                    if xc0 < NW:
                        passes.append(("x", max(xc0, c0), c1))
                    if wc0 < NW:
                        passes.append(("w", max(wc0, c0), c1))
                    passes = [p for p in passes if p[1] < p[2]]
                    for ci, (side, d0, d1) in enumerate(passes):
                        for k in range(0, KT, 2):
                            if side == "m":
                                lhsT = xht[mat][:, k : k + 2, rows]
                                rhs = w[:, k : k + 2, d0:d1]
                            elif side == "x":
                                lhsT = xlt[mat][:, k : k + 2, rows]
                                rhs = w[:, k : k + 2, d0:d1]
                            else:
                                lhsT = xht[mat][:, k : k + 2, rows]
                                rhs = wlt[:, k : k + 2, d0 - wc0 : d1 - wc0]
                            nc.tensor.matmul(
                                ps[:, d0:d1],
                                lhsT=lhsT,
                                rhs=rhs,
                                start=(ci == 0 and k == 0),
                                stop=(ci == len(passes) - 1 and k == KT - 2),
                                perf_mode=DR,
                            )

                for m in ms:
                    rows = slice(m * 128, (m + 1) * 128)
                    if split_ag and mat == 2:
                        # tail latency trick: a-gate columns get their own
                        # PSUM group that completes one pass-set early, so
                        # the tanh (longest downstream pole) runs while the
                        # sigmoid columns still accumulate
                        psA = psmm.tile([128, NW], F32, tag="mm", name="mm")
                        emit_group(psA, rows, 384, NW, mat)
                        ps = psmm.tile([128, NW], F32, tag="mm", name="mm")
                        emit_group(ps, rows, 0, 384, mat)
                        pa[m] = pa[m] + (psA,)
                    else:
                        ps = psmm.tile([128, NW], F32, tag="mm", name="mm")
                        emit_group(ps, rows, 0, NW, mat)

                    if mat == 0:
                        p1s[m] = pp.tile([128, NW], F32, tag=f"p1_{m}",
                                         name=f"p1_{m}")
                        nc.scalar.copy(p1s[m][:], ps[:])
                    elif mat == 1:
                        p2s[m] = pp.tile([128, NW], F32, tag=f"p2_{m}",
                                         name=f"p2_{m}")
                        nc.scalar.copy(p2s[m][:], ps[:])
                        # ---- epilogue phase A: everything that only
                        # needs P1/P2 (not P3) — overlaps the P3 matmuls.
                        # Gate cols: o=[0:128] f=[128:256] i=[256:384]
                        # a=[384:512]. PSUM holds SW*z.
                        obw = slice(ob * NW, (ob + 1) * NW)
                        p1, p2 = p1s[m], p2s[m]
                        zr = epp.tile([128, NW], F32, tag="zra", name="zra")
                        nc.vector.tensor_sub(zr[:], p1[:], p2[:])
                        nc.gpsimd.tensor_add(zr[:], zr[:], bias_r[:, obw])
                        gr = epp.tile([128, NW], F32, tag=f"gr_{m}",
                                      name=f"gr_{m}", bufs=1)
                        nc.scalar.activation(gr[:, 0:384], zr[:, 0:384],
                                             AFT.Sigmoid, scale=1.0 / SP)
                        nc.scalar.activation(gr[:, 384:512], zr[:, 384:512],
                                             AFT.Tanh, scale=1.0 / SP)
                        # q = p1 + p2 - bias_i (scaled): phase B does
                        # zi = P3 - q in one DVE op off the bias path
                        q = epp.tile([128, NW], F32, tag=f"q_{m}",
                                     name=f"q_{m}", bufs=1)
                        nc.vector.tensor_add(q[:], p1[:], p2[:])
                        nc.gpsimd.tensor_sub(q[:], q[:], bias_i[:, obw])
                        cr = ctile[m][:, ocols]
                        ci = ctile[m][:, oicols]
                        ir_ = gr[:, 0:128]
                        fr = gr[:, 128:256]
                        orr = gr[:, 256:384]
                        ar = gr[:, 384:512]
                        u1 = prodp.tile([128, 128], F32, tag=f"u1_{m}",
                                        name=f"u1_{m}", bufs=1)
                        u4 = prodp.tile([128, 128], F32, tag=f"u4_{m}",
                                        name=f"u4_{m}", bufs=1)
                        v1 = prodp.tile([128, 128], F32, tag=f"v1_{m}",
                                        name=f"v1_{m}", bufs=1)
                        nc.vector.tensor_mul(u1[:], cr, fr)
                        nc.vector.tensor_mul(u4[:], ci, fr)
                        nc.vector.tensor_mul(v1[:], ar, ir_)
                        pa[m] = (gr, q, u1, u4, v1)
                    else:
                        # ---- epilogue phase B for (ob, m): ps holds P3 ----
                        if split_ag:
                            gr, q, u1, u4, v1, psA = pa[m]
                        else:
                            gr, q, u1, u4, v1 = pa[m]
                            psA = ps
                        zi = epp.tile([128, NW], F32, tag="zi", name="zi")
                        gi = epp.tile([128, NW], F32, tag="gi", name="gi")
                        # a-gate first: with split_ag its group closed while
                        # the sigmoid columns still accumulate, and its tanh
                        # is the longest downstream dependency
                        nc.vector.tensor_sub(zi[:, 384:512], psA[:, 384:512],
                                             q[:, 384:512])
                        nc.scalar.activation(gi[:, 384:512], zi[:, 384:512],
                                             AFT.Tanh, scale=1.0 / SP)
                        nc.vector.tensor_sub(zi[:, 0:384], ps[:, 0:384],
                                             q[:, 0:384])
                        nc.scalar.activation(gi[:, 0:384], zi[:, 0:384],
                                             AFT.Sigmoid, scale=1.0 / SP)
                        cr = ctile[m][:, ocols]
                        ci = ctile[m][:, oicols]
                        ii_ = gi[:, 0:128]
                        fi = gi[:, 128:256]
                        oi = gi[:, 256:384]
                        ai = gi[:, 384:512]
                        ir_ = gr[:, 0:128]
                        orr = gr[:, 256:384]
                        ar = gr[:, 384:512]
                        u2 = prodp.tile([128, 128], F32, tag="u2", name="u2")
                        u3 = prodp.tile([128, 128], F32, tag="u3", name="u3")
                        v2 = prodp.tile([128, 128], F32, tag="v2", name="v2")
                        v3 = prodp.tile([128, 128], F32, tag="v3", name="v3")
                        v4 = prodp.tile([128, 128], F32, tag="v4", name="v4")
                        nc.vector.tensor_mul(u2[:], ci, fi)
                        nc.vector.tensor_mul(u3[:], cr, fi)
                        nc.gpsimd.tensor_mul(v2[:], ai, ii_)
                        nc.gpsimd.tensor_mul(v3[:], ar, ii_)
                        nc.vector.tensor_mul(v4[:], ai, ir_)
                        cfr = prodp.tile([128, 128], F32, tag="cfr", name="cfr")
                        cfi = prodp.tile([128, 128], F32, tag="cfi", name="cfi")
                        air = prodp.tile([128, 128], F32, tag="air", name="air")
                        aii = prodp.tile([128, 128], F32, tag="aii", name="aii")
                        nc.vector.tensor_sub(cfr[:], u1[:], u2[:])
                        nc.vector.tensor_add(cfi[:], u3[:], u4[:])
                        nc.gpsimd.tensor_sub(air[:], v1[:], v2[:])
                        nc.gpsimd.tensor_add(aii[:], v3[:], v4[:])
                        # c_t in bf16 (output dtype): tanh reads bf16 fine
                        ctr = prodp.tile([128, 128], BF16, tag="ctr", name="ctr")
                        cti = prodp.tile([128, 128], BF16, tag="cti", name="cti")
                        nc.vector.tensor_add(ctr[:], cfr[:], air[:])
                        nc.vector.tensor_add(cti[:], cfi[:], aii[:])
                        tr = prodp.tile([128, 128], F32, tag="tr", name="tr")
                        ti = prodp.tile([128, 128], F32, tag="ti", name="ti")
                        nc.scalar.activation(tr[:], ctr[:], AFT.Tanh)
                        nc.scalar.activation(ti[:], cti[:], AFT.Tanh)
                        htr = prodp.tile([128, 128], BF16, tag="htr", name="htr")
                        hti = prodp.tile([128, 128], BF16, tag="hti", name="hti")
                        w1 = prodp.tile([128, 128], F32, tag="w1", name="w1")
                        w2 = prodp.tile([128, 128], F32, tag="w2", name="w2")
                        w3 = prodp.tile([128, 128], F32, tag="w3", name="w3")
                        w4 = prodp.tile([128, 128], F32, tag="w4", name="w4")
                        # real half on DVE, imag half on GPSIMD in parallel
                        nc.vector.tensor_mul(w1[:], orr, tr[:])
                        nc.vector.tensor_mul(w2[:], oi, ti[:])
                        nc.vector.tensor_sub(htr[:], w1[:], w2[:])
                        nc.gpsimd.tensor_mul(w3[:], orr, ti[:])
                        nc.gpsimd.tensor_mul(w4[:], oi, tr[:])
                        nc.gpsimd.tensor_add(hti[:], w3[:], w4[:])
                        nc.sync.dma_start(out=h_out[rows, ocols], in_=htr[:])
                        nc.gpsimd.dma_start(out=h_out[rows, oicols], in_=hti[:])
                        nc.gpsimd.dma_start(out=c_out[rows, ocols], in_=ctr[:])
                        nc.sync.dma_start(out=c_out[rows, oicols], in_=cti[:])

            # PE stream order: X1 hi/lo loads, then the first matmul block
            # (only needs X1), then X2/X3 loads while that block runs.
            d1 = emit_xloads(xh[0], xht[0], first_pair_fast=True)
            if xlt[0] is not None:
                d1 += emit_xloads(xl[0], xlt[0])
            emit_mat(0, 0)
            d2 = []
            for m in (1, 2):
                d2 += emit_xloads(xh[m], xht[m])
                if xlt[m] is not None:
                    d2 += emit_xloads(xl[m], xlt[m])
            # host-broadcast bias tiles ride the SWDGE ring after all
            # x/h loads (first use is the phase-A gpsimd adds)
            for _bt, _bi in ((bias_r, 0), (bias_i, 1)):
                d = nc.gpsimd.dma_start(out=_bt[:], in_=bbc[_bi])
                add_dep_helper(d.ins, d2[-1].ins, sync=False,
                               reason="bias loads after x loads")
            # c tiles ride the SWDGE ring after all x loads (first use of
            # c is the phase-A products)
            for m in range(MT):
                t = cresp.tile([128, 2 * H], BF16, tag=f"c_m{m}", name=f"c_m{m}")
                d = nc.gpsimd.dma_start(out=t[:], in_=cx[m * 128 : (m + 1) * 128, :])
                add_dep_helper(d.ins, d2[-1].ins, sync=False,
                               reason="c loads after x loads")
                ctile.append(t)
            emit_mat(0, 1)
            emit_mat(0, 2)
            for ob in range(1, OB):
                for mat in range(3):
                    if ob == OB - 1 and mat == 2:
                        # split the final P3 so only two phase-B chains
                        # drain after the last matmul
                        wt = (wp.tile([128, KT, NW], F8, tag="wslab",
                                      name="wslab"),
                              wlp.tile([128, KT, NW - CORR_CFG[2][1]], F8,
                                       tag="wlslab", name="wlslab")
                              if CORR_CFG[2][1] < NW else None)
                        nc.sync.dma_start(
                            out=wt[0].rearrange("p kt c -> p (kt c)"),
                            in_=wh[2, ob].rearrange("p kt c -> p (kt c)"),
                        )
                        if wt[1] is not None:
                            nc.sync.dma_start(
                                out=wt[1].rearrange("p kt c -> p (kt c)"),
                                in_=wl[2][ob].rearrange("p kt c -> p (kt c)"),
                            )
                        emit_mat(ob, mat, ms=range(0, 2), wtiles=wt)
                        emit_mat(ob, mat, ms=range(2, MT), wtiles=wt,
                                 split_ag=True)
                    else:
                        emit_mat(ob, mat)
    return nc


_NC_CACHE = None


def _get_program():
    global _NC_CACHE
    if _NC_CACHE is None:
        nc = _build_program()
        fixed = _split_multiwait_json(nc.to_json_bytes())
        nc.to_json_bytes = lambda: fixed
        _NC_CACHE = nc
    return _NC_CACHE


F8NP = ml_dtypes.float8_e4m3


def _split8(a):
    a = a * SX
    ah = a.astype(F8NP)
    al = (a - ah.astype(np.float32)).astype(F8NP)
    return ah, al


def _pack_weights(Uw_r, Uw_i, Ub_r, Ub_i, Ww_r, Ww_i, Wb_r, Wb_i):
    GORD = [1, 0, 3, 2]  # column blocks [i, f, o, a]: sigmoid trio is
    # [0:384] (one act call), tanh at [384:512]; correction suffixes cover
    # gates in sensitivity order a > o > f > i.

    def interleave_cols(Wg):  # [2048, G, H] -> [2048, GH]
        return (
            Wg.reshape(K, G, OB, 128)[:, GORD]
            .transpose(0, 2, 1, 3)
            .reshape(K, G * H)
        )

    Wr = np.concatenate(
        [np.transpose(Uw_r, (2, 0, 1)), np.transpose(Ww_r, (2, 0, 1))], axis=0
    )
    Wi = np.concatenate(
        [np.transpose(Uw_i, (2, 0, 1)), np.transpose(Ww_i, (2, 0, 1))], axis=0
    )
    W1 = interleave_cols(Wr) * SW
    W2 = interleave_cols(Wi) * SW
    W3 = W1 + W2
    Wall = np.stack([W1, W2, W3])  # [3, 2048, 4096] f32, pre-scaled

    def slabify(Wm, cols):  # [2048, ncols] -> [ob, 128, KT, ncols_per_ob]
        ncpo = cols
        return (
            Wm.reshape(KT, 128, OB, ncpo)
            .transpose(2, 1, 0, 3)
        )

    whs = []
    wls = []
    for m in range(3):
        Wh8 = Wall[m].astype(F8NP)
        Wl8 = (Wall[m] - Wh8.astype(np.float32)).astype(F8NP)
        whs.append(
            Wh8.reshape(KT, 128, OB, NW).transpose(2, 1, 0, 3)
        )
        wc0 = CORR_CFG[m][1]
        if wc0 < NW:
            wlm = (
                Wl8.reshape(KT, 128, OB, NW)
                .transpose(2, 1, 0, 3)[:, :, :, wc0:]
            )
            wls.append(np.ascontiguousarray(wlm))
        else:
            wls.append(None)
    wh = np.ascontiguousarray(np.stack(whs))  # [3, OB, 128, KT, NW]

    def interleave_bias(b):  # [G, H] -> [GH] interleaved, pre-scaled
        return b.reshape(G, OB, 128)[GORD].transpose(1, 0, 2).reshape(G * H)

    br = interleave_bias((Ub_r + Wb_r) * SP)
    bi = interleave_bias((Ub_i + Wb_i) * SP)
    bbc = np.ascontiguousarray(np.broadcast_to(
        np.stack([br, bi])[:, None, :], (2, 128, G * H)
    ).astype(ml_dtypes.bfloat16))
    return wh, wls, bbc


def kernel(input, h_x, c_x, Uw_r, Uw_i, Ub_r, Ub_i, Ww_r, Ww_i, Wb_r, Wb_i,
           _trace=False):
    input = np.asarray(input, dtype=np.float32)
    h_x = np.asarray(h_x, dtype=np.float32)
    c_x = np.asarray(c_x, dtype=np.float32)
    wh, wls, bpk = _pack_weights(
        np.asarray(Uw_r, np.float32), np.asarray(Uw_i, np.float32),
        np.asarray(Ub_r, np.float32), np.asarray(Ub_i, np.float32),
        np.asarray(Ww_r, np.float32), np.asarray(Ww_i, np.float32),
        np.asarray(Wb_r, np.float32), np.asarray(Wb_i, np.float32),
    )

    X1 = np.concatenate([input[:, :IN], h_x[:, :H]], axis=1)
    X2 = np.concatenate([input[:, IN:], h_x[:, H:]], axis=1)
    X3 = X1 + X2
    xparts = [_split8(X) for X in (X1, X2, X3)]

    in_maps = []
    for c in range(NCORES):
        rows = slice(c * BL, (c + 1) * BL)
        im = {
            "cx": np.ascontiguousarray(c_x[rows].astype(ml_dtypes.bfloat16)),
            "wh": wh,
            "bbc": bpk,
        }
        for m in range(3):
            xhm, xlm = xparts[m]
            im[f"x{m}h"] = np.ascontiguousarray(
                xhm[rows].T.reshape(KT, 128, BL)
            )
            if CORR_CFG[m][0] < NW:
                im[f"x{m}l"] = np.ascontiguousarray(
                    xlm[rows].T.reshape(KT, 128, BL)
                )
            if wls[m] is not None:
                im[f"wl{m}"] = wls[m]
        in_maps.append(im)

    nc = _get_program()
    res = run_bass_kernel_spmd(
        nc, in_maps, core_ids=list(range(NCORES)), trace=_trace
    )
    h_t = np.concatenate(
        [res.results[i]["h_out"].astype(np.float32) for i in range(NCORES)],
        axis=0,
    )
    c_t = np.concatenate(
        [res.results[i]["c_out"].astype(np.float32) for i in range(NCORES)],
        axis=0,
    )
    if _trace:
        kernel._last_results = res
    return h_t, c_t


# revision 26
# speedup vs baseline: 1.5733x; 1.0064x over previous
"""Complex LSTM cell (CLSTMCell) Trainium2 kernel — fp8 DoubleRow edition.

Full inputs in, full outputs out. Data-parallel over batch: B=4096 rows
sharded 512/core across 8 NeuronCores; the weight matrices are replicated
(host pre-packed into a matmul-friendly fp8 layout).

Math: with X1=[xr|hr], X2=[xi|hi] ([B,2048]) and W1=[Ur;Wr], W2=[Ui;Wi]
([2048,4096]), the complex gate projection is computed via Karatsuba:
  P1 = X1@W1, P2 = X2@W2, P3 = (X1+X2)@(W1+W2)
  Zr = P1 - P2 (+ br),  Zi = P3 - P1 - P2 (+ bi)
i.e. 3 real matmuls instead of 4 (25% FLOP cut).

Matmuls run in fp8-e4m3 with MatmulPerfMode.DoubleRow (two k-subtiles per
instruction at 0.5 cycles/row = 4x bf16 PE throughput). fp8's 3-bit
mantissa alone is too coarse (rel err ~5e-2 > 2e-2 gate), so each operand
is split hi/lo: X*4 = Xh + Xl, W*1024 = Wh + Wl (all four parts fp8;
the 4096 product scale folds out via the activation `scale` arg). The
product is corrected per gate-column block:
  P = Xh@Wh [+ Xl@Wh (X-corr)] [+ Xh@Wl (W-corr)]
Correction column sets are per-mat suffixes of the gate order [i,f,o,a]
(CORR_CFG below), chosen by offline error search: less-sensitive gates
skip corrections so only the columns that matter pay the extra DR passes.

Weight columns are interleaved as c = oblk*512 + gate*128 + (o % 128)
with gate order [i,f,o,a], so each N=512 matmul block contains all 4
gates for one 128-wide o slice, letting the cell update complete
per-block with no cross-block buffering.
"""

import sys

for _p in ("/opt/trn_rl_repo",):
    if _p not in sys.path:
        sys.path.insert(0, _p)

import numpy as np
import ml_dtypes

import concourse.bass as bass
import concourse.mybir as mybir
from concourse.bass_utils import run_bass_kernel_spmd
from concourse.tile import TileContext, add_dep_helper

F32 = mybir.dt.float32
BF16 = mybir.dt.bfloat16
F8 = mybir.dt.float8e4
AFT = mybir.ActivationFunctionType
DR = mybir.MatmulPerfMode.DoubleRow

B = 4096
IN = 1024
H = 1024
G = 4
NCORES = 8
BL = B // NCORES          # 512 batch rows per core
MT = BL // 128            # 4 m-tiles per core
K = 2 * IN                # 2048 contraction dim (x|h concat)
KT = K // 128             # 16 k-tiles
OB = H // 128             # 8 o-blocks
NW = G * 128              # 512 matmul N (all gates for one o-block)
SX = 4.0                  # x-side pre-scale
SW = 1024.0               # weight pre-scale
SP = SX * SW              # product scale, folded out via activation scale
# Scales keep all four fp8 operand classes (Xh, Xl, Wh, Wl) out of e4m3's
# subnormal range (hi parts sigma ~4 / ~22; residuals sigma ~0.07 / ~0.4,
# vs tiny=0.0156), so correctness survives even if the PE flushes fp8
# subnormals (the interpreter doesn't, hardware behavior unverified).

# Gate order within each 512-wide o-block: [i, f, o, a] (measured output
# sensitivity to z-error: a > o > f > i).
# Column slices: i=[0:128] f=[128:256] o=[256:384] a=[384:512].
# Per-mat split of each o-block's columns into a mixed-correction prefix
# [0:MIX_END] and a full-correction suffix [MIX_END:512]:
#   mixed:  P = Xh@(Wh/2) + A@B, A = fp8(Xh/2 + Xl), B = fp8(Wh + 2*Wl)
#           = XhWh + XlWh + XhWl + O(XlWl): both-side correction at half
#           the residual of a one-side pass, for ONE extra DR pass
#   full:   P = Xh@Wh + Xl@Wh + Xh@Wl (TWO extra passes, ~exact)
# (Wh/2 is exact in fp8 — exponent decrement.) Offline error search puts
# gates {i,f,o} of P1/P2 and {i} of P3 on mixed, the rest on full:
# measured ~1.4e-2 hw rel err vs the 2e-2 gate.
MIX_END = [384, 384, 128]


def _split_multiwait_json(raw: bytes) -> bytes:
    """The walrus build in this container accepts at most one sem wait
    per instruction; Tile's scheduler packs several. Split the extras
    into preceding wait-only EventSemaphore instructions on the same
    engine (same semantics: the sequencer blocks on each in order)."""
    import orjson

    m = orjson.loads(raw)
    ctr = 0
    for fn in m["functions"]:
        for bb in fn["blocks"]:
            out = []
            for ins in bb["instructions"]:
                si = ins.get("sync_info")
                waits = si.get("on_wait") if si else None
                if waits and len(waits) > 1:
                    for w in waits[:-1]:
                        ctr += 1
                        nop = {
                            "engine": ins["engine"],
                            "ins": [],
                            "outs": [],
                            "name": f"{ins['name']}_sw{ctr}",
                            "opcode": "EventSemaphore",
                            "sync_info": {"on_update": [], "on_wait": [w]},
                        }
                        if "debug" in ins:
                            nop["debug"] = ins["debug"]
                        out.append(nop)
                    si["on_wait"] = [waits[-1]]
                out.append(ins)
            bb["instructions"] = out
    return orjson.dumps(m)


def _build_program():
    nc = bass.Bass()

    # x-side: hi/lo fp8 for X1, X2, X3, pre-transposed on host: [KT,128,BL]
    xh = [nc.dram_tensor(f"x{m}h", [KT, 128, BL], F8, kind="ExternalInput")
          for m in range(3)]
    xl = [nc.dram_tensor(f"x{m}l", [KT, 128, BL], F8, kind="ExternalInput")
          for m in range(3)]
    xa = [nc.dram_tensor(f"x{m}a", [KT, 128, BL], F8, kind="ExternalInput")
          for m in range(3)]
    cx = nc.dram_tensor("cx", [BL, 2 * H], BF16, kind="ExternalInput")
    # combined weight slab per (mat, ob): [Wh(512, mixed cols pre-halved)
    # | B(MIX_END) | Wl(512-MIX_END)] = uniform 1024 columns; column c's
    # correction operand (B for mixed cols, Wl for full cols) sits at
    # 512+c, so one DMA feeds all four pass types k-progressively.
    wcomb = nc.dram_tensor("wcomb", [3, OB, 128, KT, 2 * NW], F8,
                           kind="ExternalInput")
    bbc = nc.dram_tensor("bbc", [2, 128, G * H], BF16, kind="ExternalInput")
    h_out = nc.dram_tensor("h_out", [BL, 2 * H], BF16, kind="ExternalOutput")
    c_out = nc.dram_tensor("c_out", [BL, 2 * H], BF16, kind="ExternalOutput")

    with TileContext(nc) as tc:
        with (
            tc.tile_pool(name="const", bufs=2) as constp,
            tc.tile_pool(name="cres", bufs=2) as cresp,
            tc.tile_pool(name="xt", bufs=1) as xtp,
            tc.tile_pool(name="w", bufs=3) as wp,
            tc.tile_pool(name="pp", bufs=1) as pp,
            tc.tile_pool(name="ep", bufs=2) as epp,
            tc.tile_pool(name="prod", bufs=2) as prodp,
            tc.tile_pool(name="ps_mm", bufs=6, space="PSUM") as psmm,
        ):
            # per-ob bias ([128,512] slices) and c ([128,2,128] per m-tile)
            # land right before their o-block — keeps 24KB/partition free
            # so the weight-slab pool can triple-buffer
            obres = {}

            def load_ob_resources(ob):
                brt = constp.tile([128, NW], BF16, tag="bias_r", name="bias_r")
                bit = constp.tile([128, NW], BF16, tag="bias_i", name="bias_i")
                obw = slice(ob * NW, (ob + 1) * NW)
                nc.gpsimd.dma_start(out=brt[:], in_=bbc[0][:, obw])
                nc.gpsimd.dma_start(out=bit[:], in_=bbc[1][:, obw])
                cts = []
                for m in range(MT):
                    t = cresp.tile([128, 2, 128], BF16, tag=f"c_m{m}",
                                   name=f"c_m{m}")
                    src = cx[m * 128 : (m + 1) * 128, :].rearrange(
                        "r (two h) -> r two h", two=2
                    )[:, :, ob * 128 : (ob + 1) * 128]
                    nc.gpsimd.dma_start(out=t[:], in_=src)
                    cts.append(t)
                obres[ob] = (brt, bit, cts)

            # X tiles: one [128, KT, BL] tile per tensor; DR slices
            # [:, 2k:2k+2, m*128:(m+1)*128] need k-pairs adjacent in dim1.
            xht = [xtp.tile([128, KT, BL], F8, tag=f"xh{m}", name=f"xh{m}")
                   for m in range(3)]
            xlt = [xtp.tile([128, KT, BL], F8, tag=f"xl{m}", name=f"xl{m}")
                   for m in range(3)]
            att = [xtp.tile([128, KT, BL], F8, tag=f"xa{m}", name=f"xa{m}")
                   for m in range(3)]

            def emit_xloads(src, dst, quarters=4, first_pair_fast=False):
                # dram [KT,128,BL] -> sbuf [128, KT, BL] in `quarters` DMAs;
                # first_pair_fast peels k-tiles 0-1 onto the idle DVE HWDGE
                # ring so the very first matmul unblocks sooner (ACT is idle
                # at t=0; DVE cannot issue DMAs in this build).
                dmas = []
                kq = KT // quarters
                for q in range(quarters):
                    eng = nc.scalar if first_pair_fast else nc.gpsimd
                    if first_pair_fast and q == 0:
                        # peel k-tiles 0-1 so the first matmul unblocks at
                        # the DGE-init floor
                        dmas.append(nc.scalar.dma_start(
                            out=dst[:, 0:2, :],
                            in_=src[0:2].rearrange("kt p b -> p kt b"),
                        ))
                        dmas.append(nc.scalar.dma_start(
                            out=dst[:, 2:4, :],
                            in_=src[2:4].rearrange("kt p b -> p kt b"),
                        ))
                        continue
                    dmas.append(eng.dma_start(
                        out=dst[:, q * kq : (q + 1) * kq, :],
                        in_=src[q * kq : (q + 1) * kq].rearrange(
                            "kt p b -> p kt b"
                        ),
                    ))
                return dmas

            p1s_all = {ob: [None] * MT for ob in range(OB)}
            p2s_all = {ob: [None] * MT for ob in range(OB)}
            pa = [None] * MT

            def emit_mat(ob, mat, ms=None, wtiles=None, split_ag=False,
                         defer_corr=False):
                ocols = slice(ob * 128, (ob + 1) * 128)
                oicols = slice(H + ob * 128, H + (ob + 1) * 128)
                p1s = p1s_all[ob]
                p2s = p2s_all[ob]
                if ms is None:
                    ms = range(MT)
                if mat == 0 and ob not in obres:
                    load_ob_resources(ob)
                if wtiles is None:
                    wtiles = load_slabs(ob, mat)
                return emit_mat_body(ob, mat, ms, wtiles, ocols, oicols,
                                     p1s, p2s, split_ag, defer_corr)

            def load_slabs(ob, mat):
                w = wp.tile([128, KT, 2 * NW], F8, tag="wslab", name="wslab")
                wsrc = wcomb[mat, ob].rearrange("p kt c -> p (kt c)")
                wdst = w.rearrange("p kt c -> p (kt c)")
                if ob == 0 and mat == 0:
                    # chunk the very first slab by k-pairs so every pass
                    # type starts as soon as its k-tiles land
                    qr = KT * 2 * NW // 8
                    for _q in range(8):
                        nc.sync.dma_start(
                            out=wdst[:, _q * qr : (_q + 1) * qr],
                            in_=wsrc[:, _q * qr : (_q + 1) * qr],
                        )
                elif ob == 0 and mat == 1:
                    half = KT * 2 * NW // 2
                    nc.sync.dma_start(out=wdst[:, :half], in_=wsrc[:, :half])
                    nc.sync.dma_start(out=wdst[:, half:], in_=wsrc[:, half:])
                else:
                    nc.sync.dma_start(out=wdst[:], in_=wsrc)
                return w

            def emit_mat_body(ob, mat, ms, wtiles, ocols, oicols,
                              p1s, p2s, split_ag=False, defer_corr=False):
                w = wtiles
                me = MIX_END[mat]

                def emit_group(ps, rows, c0, c1, mat, phase="all"):
                    # one accumulation group covering columns [c0:c1]:
                    # main everywhere, mixed pass on [c0:me), full hi/lo
                    # passes on [me:c1). phase="main"/"corr" emits a
                    # subset (used while the slab is still streaming in).
                    passes = [("m", c0, c1)]
                    if c1 > me:
                        passes.append(("x", max(c0, me), c1))
                        passes.append(("w", max(c0, me), c1))
                    if c0 < me:
                        passes.append(("a", c0, min(c1, me)))
                    if phase == "main":
                        passes = passes[:1]
                    elif phase == "corr":
                        passes = [("_skip", c0, c1)] + passes[1:]
                    for ci, (side, d0, d1) in enumerate(passes):
                        if side == "_skip":
                            continue
                        for k in range(0, KT, 2):
                            if side == "m":
                                lhsT = xht[mat][:, k : k + 2, rows]
                                rhs = w[:, k : k + 2, d0:d1]
                            elif side == "a":
                                lhsT = att[mat][:, k : k + 2, rows]
                                rhs = w[:, k : k + 2, NW + d0 : NW + d1]
                            elif side == "x":
                                lhsT = xlt[mat][:, k : k + 2, rows]
                                rhs = w[:, k : k + 2, d0:d1]
                            else:
                                lhsT = xht[mat][:, k : k + 2, rows]
                                rhs = w[:, k : k + 2, NW + d0 : NW + d1]
                            nc.tensor.matmul(
                                ps[:, d0:d1],
                                lhsT=lhsT,
                                rhs=rhs,
                                start=(ci == 0 and k == 0),
                                stop=(phase != "main" and
                                      ci == len(passes) - 1 and k == KT - 2),
                                perf_mode=DR,
                            )

                deferred = {}
                if defer_corr:
                    for m in ms:
                        rows = slice(m * 128, (m + 1) * 128)
                        ps = psmm.tile([128, NW], F32, tag="mm", name="mm")
                        emit_group(ps, rows, 0, NW, mat, phase="main")
                        deferred[m] = ps
                for m in ms:
                    rows = slice(m * 128, (m + 1) * 128)
                    if m in deferred:
                        ps = deferred[m]
                        emit_group(ps, rows, 0, NW, mat, phase="corr")
                    elif split_ag and mat == 2:
                        # tail latency trick: a-gate columns get their own
                        # PSUM group that completes one pass-set early, so
                        # the tanh (longest downstream pole) runs while the
                        # sigmoid columns still accumulate
                        psA = psmm.tile([128, NW], F32, tag="mm", name="mm")
                        emit_group(psA, rows, 384, NW, mat)
                        ps = psmm.tile([128, NW], F32, tag="mm", name="mm")
                        emit_group(ps, rows, 0, 384, mat)
                        pa[m] = pa[m] + (psA,)
                    else:
                        ps = psmm.tile([128, NW], F32, tag="mm", name="mm")
                        emit_group(ps, rows, 0, NW, mat)

                    if mat == 0:
                        p1s[m] = pp.tile([128, NW], F32, tag=f"p1_{m}",
                                         name=f"p1_{m}")
                        nc.scalar.copy(p1s[m][:], ps[:])
                    elif mat == 1:
                        p2s[m] = pp.tile([128, NW], F32, tag=f"p2_{m}",
                                         name=f"p2_{m}")
                        nc.scalar.copy(p2s[m][:], ps[:])
                        # ---- epilogue phase A: everything that only
                        # needs P1/P2 (not P3) — overlaps the P3 matmuls.
                        # Gate cols: o=[0:128] f=[128:256] i=[256:384]
                        # a=[384:512]. PSUM holds SW*z.
                        brt, bit, cts = obres[ob]
                        p1, p2 = p1s[m], p2s[m]
                        zr = epp.tile([128, NW], F32, tag="zra", name="zra")
                        nc.vector.tensor_sub(zr[:], p1[:], p2[:])
                        nc.gpsimd.tensor_add(zr[:], zr[:], brt[:])
                        gr = epp.tile([128, NW], F32, tag=f"gr_{m}",
                                      name=f"gr_{m}", bufs=1)
                        nc.scalar.activation(gr[:, 0:384], zr[:, 0:384],
                                             AFT.Sigmoid, scale=1.0 / SP)
                        nc.scalar.activation(gr[:, 384:512], zr[:, 384:512],
                                             AFT.Tanh, scale=1.0 / SP)
                        # q = p1 + p2 - bias_i (scaled): phase B does
                        # zi = P3 - q in one DVE op off the bias path
                        q = epp.tile([128, NW], F32, tag=f"q_{m}",
                                     name=f"q_{m}", bufs=1)
                        nc.vector.tensor_add(q[:], p1[:], p2[:])
                        nc.gpsimd.tensor_sub(q[:], q[:], bit[:])
                        cr = cts[m][:, 0, :]
                        ci = cts[m][:, 1, :]
                        ir_ = gr[:, 0:128]
                        fr = gr[:, 128:256]
                        orr = gr[:, 256:384]
                        ar = gr[:, 384:512]
                        u1 = prodp.tile([128, 128], F32, tag=f"u1_{m}",
                                        name=f"u1_{m}", bufs=1)
                        u4 = prodp.tile([128, 128], F32, tag=f"u4_{m}",
                                        name=f"u4_{m}", bufs=1)
                        v1 = prodp.tile([128, 128], F32, tag=f"v1_{m}",
                                        name=f"v1_{m}", bufs=1)
                        nc.vector.tensor_mul(u1[:], cr, fr)
                        nc.vector.tensor_mul(u4[:], ci, fr)
                        nc.vector.tensor_mul(v1[:], ar, ir_)
                        pa[m] = (gr, q, u1, u4, v1)
                    else:
                        # ---- epilogue phase B for (ob, m): ps holds P3 ----
                        if split_ag:
                            gr, q, u1, u4, v1, psA = pa[m]
                        else:
                            gr, q, u1, u4, v1 = pa[m]
                            psA = ps
                        zi = epp.tile([128, NW], F32, tag="zi", name="zi")
                        gi = epp.tile([128, NW], F32, tag="gi", name="gi")
                        # a-gate first: with split_ag its group closed while
                        # the sigmoid columns still accumulate, and its tanh
                        # is the longest downstream dependency
                        nc.vector.tensor_sub(zi[:, 384:512], psA[:, 384:512],
                                             q[:, 384:512])
                        nc.scalar.activation(gi[:, 384:512], zi[:, 384:512],
                                             AFT.Tanh, scale=1.0 / SP)
                        nc.vector.tensor_sub(zi[:, 0:384], ps[:, 0:384],
                                             q[:, 0:384])
                        nc.scalar.activation(gi[:, 0:384], zi[:, 0:384],
                                             AFT.Sigmoid, scale=1.0 / SP)
                        cr = obres[ob][2][m][:, 0, :]
                        ci = obres[ob][2][m][:, 1, :]
                        ii_ = gi[:, 0:128]
                        fi = gi[:, 128:256]
                        oi = gi[:, 256:384]
                        ai = gi[:, 384:512]
                        ir_ = gr[:, 0:128]
                        orr = gr[:, 256:384]
                        ar = gr[:, 384:512]
                        u2 = prodp.tile([128, 128], F32, tag="u2", name="u2", bufs=1)
                        u3 = prodp.tile([128, 128], F32, tag="u3", name="u3", bufs=1)
                        v2 = prodp.tile([128, 128], F32, tag="v2", name="v2", bufs=1)
                        v3 = prodp.tile([128, 128], F32, tag="v3", name="v3", bufs=1)
                        v4 = prodp.tile([128, 128], F32, tag="v4", name="v4", bufs=1)
                        nc.vector.tensor_mul(u2[:], ci, fi)
                        nc.vector.tensor_mul(u3[:], cr, fi)
                        nc.gpsimd.tensor_mul(v2[:], ai, ii_)
                        nc.gpsimd.tensor_mul(v3[:], ar, ii_)
                        nc.vector.tensor_mul(v4[:], ai, ir_)
                        cfr = prodp.tile([128, 128], F32, tag="cfr", name="cfr", bufs=1)
                        cfi = prodp.tile([128, 128], F32, tag="cfi", name="cfi", bufs=1)
                        air = prodp.tile([128, 128], F32, tag="air", name="air", bufs=1)
                        aii = prodp.tile([128, 128], F32, tag="aii", name="aii", bufs=1)
                        nc.vector.tensor_sub(cfr[:], u1[:], u2[:])
                        nc.vector.tensor_add(cfi[:], u3[:], u4[:])
                        nc.gpsimd.tensor_sub(air[:], v1[:], v2[:])
                        nc.gpsimd.tensor_add(aii[:], v3[:], v4[:])
                        # c_t in bf16 (output dtype): tanh reads bf16 fine
                        ctr = prodp.tile([128, 128], BF16, tag="ctr", name="ctr")
                        cti = prodp.tile([128, 128], BF16, tag="cti", name="cti")
                        nc.vector.tensor_add(ctr[:], cfr[:], air[:])
                        nc.vector.tensor_add(cti[:], cfi[:], aii[:])
                        tr = prodp.tile([128, 128], F32, tag="tr", name="tr")
                        ti = prodp.tile([128, 128], F32, tag="ti", name="ti")
                        nc.scalar.activation(tr[:], ctr[:], AFT.Tanh)
                        nc.scalar.activation(ti[:], cti[:], AFT.Tanh)
                        htr = prodp.tile([128, 128], BF16, tag="htr", name="htr")
                        hti = prodp.tile([128, 128], BF16, tag="hti", name="hti")
                        w1 = prodp.tile([128, 128], F32, tag="w1", name="w1")
                        w2 = prodp.tile([128, 128], F32, tag="w2", name="w2")
                        w3 = prodp.tile([128, 128], F32, tag="w3", name="w3")
                        w4 = prodp.tile([128, 128], F32, tag="w4", name="w4")
                        # real half on DVE, imag half on GPSIMD in parallel
                        nc.vector.tensor_mul(w1[:], orr, tr[:])
                        nc.vector.tensor_mul(w2[:], oi, ti[:])
                        nc.vector.tensor_sub(htr[:], w1[:], w2[:])
                        nc.gpsimd.tensor_mul(w3[:], orr, ti[:])
                        nc.gpsimd.tensor_mul(w4[:], oi, tr[:])
                        nc.gpsimd.tensor_add(hti[:], w3[:], w4[:])
                        # keep the SP ring clean for the weight-slab
                        # stream: outputs ride Pool/ACT
                        nc.scalar.dma_start(out=h_out[rows, ocols], in_=htr[:])
                        nc.gpsimd.dma_start(out=h_out[rows, oicols], in_=hti[:])
                        nc.gpsimd.dma_start(out=c_out[rows, ocols], in_=ctr[:])
                        nc.scalar.dma_start(out=c_out[rows, oicols], in_=cti[:])

            # PE stream order: X1 hi/lo loads, then the first matmul block
            # (only needs X1), then X2/X3 loads while that block runs.
            d1 = emit_xloads(xh[0], xht[0], first_pair_fast=True)
            d1 += emit_xloads(xl[0], xlt[0])
            d1 += emit_xloads(xa[0], att[0])
            emit_mat(0, 0, defer_corr=True)
            d2 = []
            for m in (1, 2):
                d2 += emit_xloads(xh[m], xht[m])
                d2 += emit_xloads(xl[m], xlt[m])
                d2 += emit_xloads(xa[m], att[m])
            emit_mat(0, 1, defer_corr=True)
            emit_mat(0, 2)
            for ob in range(1, OB):
                for mat in range(3):
                    if ob == OB - 1 and mat == 2:
                        # split the final P3 so only two phase-B chains
                        # drain after the last matmul
                        wt = load_slabs(ob, mat)
                        emit_mat(ob, mat, ms=range(0, 2), wtiles=wt)
                        emit_mat(ob, mat, ms=range(2, MT), wtiles=wt,
                                 split_ag=True)
                    else:
                        emit_mat(ob, mat)
    return nc


_NC_CACHE = None


def _get_program():
    global _NC_CACHE
    if _NC_CACHE is None:
        nc = _build_program()
        fixed = _split_multiwait_json(nc.to_json_bytes())
        nc.to_json_bytes = lambda: fixed
        _NC_CACHE = nc
    return _NC_CACHE


F8NP = ml_dtypes.float8_e4m3


def _split8(a):
    a = a * SX
    ah = a.astype(F8NP)
    ahf = ah.astype(np.float32)
    al = (a - ahf).astype(F8NP)
    aa = (0.5 * ahf + al.astype(np.float32)).astype(F8NP)
    return ah, al, aa


def _pack_weights(Uw_r, Uw_i, Ub_r, Ub_i, Ww_r, Ww_i, Wb_r, Wb_i):
    GORD = [1, 0, 3, 2]  # column blocks [i, f, o, a]: sigmoid trio is
    # [0:384] (one act call), tanh at [384:512]; correction suffixes cover
    # gates in sensitivity order a > o > f > i.

    def interleave_cols(Wg):  # [2048, G, H] -> [2048, GH]
        return (
            Wg.reshape(K, G, OB, 128)[:, GORD]
            .transpose(0, 2, 1, 3)
            .reshape(K, G * H)
        )

    Wr = np.concatenate(
        [np.transpose(Uw_r, (2, 0, 1)), np.transpose(Ww_r, (2, 0, 1))], axis=0
    )
    Wi = np.concatenate(
        [np.transpose(Uw_i, (2, 0, 1)), np.transpose(Ww_i, (2, 0, 1))], axis=0
    )
    W1 = interleave_cols(Wr) * SW
    W2 = interleave_cols(Wi) * SW
    W3 = W1 + W2
    Wall = np.stack([W1, W2, W3])  # [3, 2048, 4096] f32, pre-scaled

    def slabify(Wm, cols):  # [2048, ncols] -> [ob, 128, KT, ncols_per_ob]
        ncpo = cols
        return (
            Wm.reshape(KT, 128, OB, ncpo)
            .transpose(2, 1, 0, 3)
        )

    whs = []
    wls = []
    bss = []
    for m in range(3):
        me = MIX_END[m]
        Wh8 = Wall[m].astype(F8NP)
        Whf = Wh8.astype(np.float32)
        Wl8 = (Wall[m] - Whf).astype(F8NP)
        B8 = (Whf + 2.0 * Wl8.astype(np.float32)).astype(F8NP)
        # halve the mixed-prefix columns of Wh (exact: exponent decrement)
        Whm = Whf.reshape(K, OB, NW)
        Whm[:, :, :me] *= 0.5
        Wh8m = Whm.reshape(K, G * H).astype(F8NP)
        whm_slab = Wh8m.reshape(KT, 128, OB, NW).transpose(2, 1, 0, 3)
        b_slab = B8.reshape(KT, 128, OB, NW).transpose(2, 1, 0, 3)[:, :, :, :me]
        l_slab = Wl8.reshape(KT, 128, OB, NW).transpose(2, 1, 0, 3)[:, :, :, me:]
        whs.append(np.concatenate([whm_slab, b_slab, l_slab], axis=-1))
    wh = np.ascontiguousarray(np.stack(whs))  # [3, OB, 128, KT, 2*NW]

    def interleave_bias(b):  # [G, H] -> [GH] interleaved, pre-scaled
        return b.reshape(G, OB, 128)[GORD].transpose(1, 0, 2).reshape(G * H)

    br = interleave_bias((Ub_r + Wb_r) * SP)
    bi = interleave_bias((Ub_i + Wb_i) * SP)
    bbc = np.ascontiguousarray(np.broadcast_to(
        np.stack([br, bi])[:, None, :], (2, 128, G * H)
    ).astype(ml_dtypes.bfloat16))
    return wh, bbc


def kernel(input, h_x, c_x, Uw_r, Uw_i, Ub_r, Ub_i, Ww_r, Ww_i, Wb_r, Wb_i,
           _trace=False):
    input = np.asarray(input, dtype=np.float32)
    h_x = np.asarray(h_x, dtype=np.float32)
    c_x = np.asarray(c_x, dtype=np.float32)
    wh, bpk = _pack_weights(
        np.asarray(Uw_r, np.float32), np.asarray(Uw_i, np.float32),
        np.asarray(Ub_r, np.float32), np.asarray(Ub_i, np.float32),
        np.asarray(Ww_r, np.float32), np.asarray(Ww_i, np.float32),
        np.asarray(Wb_r, np.float32), np.asarray(Wb_i, np.float32),
    )

    X1 = np.concatenate([input[:, :IN], h_x[:, :H]], axis=1)
    X2 = np.concatenate([input[:, IN:], h_x[:, H:]], axis=1)
    X3 = X1 + X2
    xparts = [_split8(X) for X in (X1, X2, X3)]

    in_maps = []
    for c in range(NCORES):
        rows = slice(c * BL, (c + 1) * BL)
        im = {
            "cx": np.ascontiguousarray(c_x[rows].astype(ml_dtypes.bfloat16)),
            "wcomb": wh,
            "bbc": bpk,
        }
        for m in range(3):
            xhm, xlm, xam = xparts[m]
            im[f"x{m}h"] = np.ascontiguousarray(
                xhm[rows].T.reshape(KT, 128, BL)
            )
            im[f"x{m}l"] = np.ascontiguousarray(
                xlm[rows].T.reshape(KT, 128, BL)
            )
            im[f"x{m}a"] = np.ascontiguousarray(
                xam[rows].T.reshape(KT, 128, BL)
            )
        in_maps.append(im)

    nc = _get_program()
    res = run_bass_kernel_spmd(
        nc, in_maps, core_ids=list(range(NCORES)), trace=_trace
    )
    h_t = np.concatenate(
        [res.results[i]["h_out"].astype(np.float32) for i in range(NCORES)],
        axis=0,
    )
    c_t = np.concatenate(
        [res.results[i]["c_out"].astype(np.float32) for i in range(NCORES)],
        axis=0,
    )
    if _trace:
        kernel._last_results = res
    return h_t, c_t


# revision 27
# speedup vs baseline: 1.6213x; 1.0306x over previous
"""Complex LSTM cell (CLSTMCell) Trainium2 kernel — fp8 DoubleRow edition.

Full inputs in, full outputs out. Data-parallel over batch: B=4096 rows
sharded 512/core across 8 NeuronCores; the weight matrices are replicated
(host pre-packed into a matmul-friendly fp8 layout).

Math: with X1=[xr|hr], X2=[xi|hi] ([B,2048]) and W1=[Ur;Wr], W2=[Ui;Wi]
([2048,4096]), the complex gate projection is computed via Karatsuba:
  P1 = X1@W1, P2 = X2@W2, P3 = (X1+X2)@(W1+W2)
  Zr = P1 - P2 (+ br),  Zi = P3 - P1 - P2 (+ bi)
i.e. 3 real matmuls instead of 4 (25% FLOP cut).

Matmuls run in fp8-e4m3 with MatmulPerfMode.DoubleRow (two k-subtiles per
instruction at 0.5 cycles/row = 4x bf16 PE throughput). fp8's 3-bit
mantissa alone is too coarse (rel err ~5e-2 > 2e-2 gate), so each operand
is split hi/lo: X*4 = Xh + Xl, W*1024 = Wh + Wl (all four parts fp8;
the 4096 product scale folds out via the activation `scale` arg). The
product is corrected per gate-column block:
  P = Xh@Wh [+ Xl@Wh (X-corr)] [+ Xh@Wl (W-corr)]
Correction column sets are per-mat suffixes of the gate order [i,f,o,a]
(CORR_CFG below), chosen by offline error search: less-sensitive gates
skip corrections so only the columns that matter pay the extra DR passes.

Weight columns are interleaved as c = oblk*512 + gate*128 + (o % 128)
with gate order [i,f,o,a], so each N=512 matmul block contains all 4
gates for one 128-wide o slice, letting the cell update complete
per-block with no cross-block buffering.
"""

import sys

for _p in ("/opt/trn_rl_repo",):
    if _p not in sys.path:
        sys.path.insert(0, _p)

import numpy as np
import ml_dtypes

import concourse.bass as bass
import concourse.mybir as mybir
from concourse.bass_utils import run_bass_kernel_spmd
from concourse.tile import TileContext, add_dep_helper

F32 = mybir.dt.float32
BF16 = mybir.dt.bfloat16
F8 = mybir.dt.float8e4
AFT = mybir.ActivationFunctionType
DR = mybir.MatmulPerfMode.DoubleRow

B = 4096
IN = 1024
H = 1024
G = 4
NCORES = 8
BL = B // NCORES          # 512 batch rows per core
MT = BL // 128            # 4 m-tiles per core
K = 2 * IN                # 2048 contraction dim (x|h concat)
KT = K // 128             # 16 k-tiles
OB = H // 128             # 8 o-blocks
NW = G * 128              # 512 matmul N (all gates for one o-block)
SX = 4.0                  # x-side pre-scale
SW = 1024.0               # weight pre-scale
SP = SX * SW              # product scale, folded out via activation scale
# Scales keep all four fp8 operand classes (Xh, Xl, Wh, Wl) out of e4m3's
# subnormal range (hi parts sigma ~4 / ~22; residuals sigma ~0.07 / ~0.4,
# vs tiny=0.0156), so correctness survives even if the PE flushes fp8
# subnormals (the interpreter doesn't, hardware behavior unverified).

# Gate order within each 512-wide o-block: [i, f, o, a] (measured output
# sensitivity to z-error: a > o > f > i).
# Column slices: i=[0:128] f=[128:256] o=[256:384] a=[384:512].
# Per-mat split of each o-block's columns into a mixed-correction prefix
# [0:MIX_END] and a full-correction suffix [MIX_END:512]:
#   mixed:  P = Xh@(Wh/2) + A@B, A = fp8(Xh/2 + Xl), B = fp8(Wh + 2*Wl)
#           = XhWh + XlWh + XhWl + O(XlWl): both-side correction at half
#           the residual of a one-side pass, for ONE extra DR pass
#   full:   P = Xh@Wh + Xl@Wh + Xh@Wl (TWO extra passes, ~exact)
# (Wh/2 is exact in fp8 — exponent decrement.) Offline error search puts
# gates {i,f,o} of P1/P2 and {i} of P3 on mixed, the rest on full:
# measured ~1.4e-2 hw rel err vs the 2e-2 gate.
MIX_END = [384, 384, 256]


def _split_multiwait_json(raw: bytes) -> bytes:
    """The walrus build in this container accepts at most one sem wait
    per instruction; Tile's scheduler packs several. Split the extras
    into preceding wait-only EventSemaphore instructions on the same
    engine (same semantics: the sequencer blocks on each in order)."""
    import orjson

    m = orjson.loads(raw)
    ctr = 0
    for fn in m["functions"]:
        for bb in fn["blocks"]:
            out = []
            for ins in bb["instructions"]:
                si = ins.get("sync_info")
                waits = si.get("on_wait") if si else None
                if waits and len(waits) > 1:
                    for w in waits[:-1]:
                        ctr += 1
                        nop = {
                            "engine": ins["engine"],
                            "ins": [],
                            "outs": [],
                            "name": f"{ins['name']}_sw{ctr}",
                            "opcode": "EventSemaphore",
                            "sync_info": {"on_update": [], "on_wait": [w]},
                        }
                        if "debug" in ins:
                            nop["debug"] = ins["debug"]
                        out.append(nop)
                    si["on_wait"] = [waits[-1]]
                out.append(ins)
            bb["instructions"] = out
    return orjson.dumps(m)


def _build_program():
    nc = bass.Bass()

    # x-side: hi/lo fp8 for X1, X2, X3, pre-transposed on host: [KT,128,BL]
    xh = [nc.dram_tensor(f"x{m}h", [KT, 128, BL], F8, kind="ExternalInput")
          for m in range(3)]
    xl = [nc.dram_tensor(f"x{m}l", [KT, 128, BL], F8, kind="ExternalInput")
          for m in range(3)]
    xa = [nc.dram_tensor(f"x{m}a", [KT, 128, BL], F8, kind="ExternalInput")
          for m in range(3)]
    cx = nc.dram_tensor("cx", [BL, 2 * H], BF16, kind="ExternalInput")
    # combined weight slab per (mat, ob): [Wh(512, mixed cols pre-halved)
    # | B(MIX_END) | Wl(512-MIX_END)] = uniform 1024 columns; column c's
    # correction operand (B for mixed cols, Wl for full cols) sits at
    # 512+c, so one DMA feeds all four pass types k-progressively.
    wcomb = nc.dram_tensor("wcomb", [3, OB, 128, KT, 2 * NW], F8,
                           kind="ExternalInput")
    bbc = nc.dram_tensor("bbc", [2, 128, G * H], BF16, kind="ExternalInput")
    h_out = nc.dram_tensor("h_out", [BL, 2 * H], BF16, kind="ExternalOutput")
    c_out = nc.dram_tensor("c_out", [BL, 2 * H], BF16, kind="ExternalOutput")

    with TileContext(nc) as tc:
        with (
            tc.tile_pool(name="const", bufs=2) as constp,
            tc.tile_pool(name="cres", bufs=2) as cresp,
            tc.tile_pool(name="xt", bufs=1) as xtp,
            tc.tile_pool(name="w", bufs=3) as wp,
            tc.tile_pool(name="pp", bufs=1) as pp,
            tc.tile_pool(name="ep", bufs=2) as epp,
            tc.tile_pool(name="prod", bufs=2) as prodp,
            tc.tile_pool(name="ps_mm", bufs=6, space="PSUM") as psmm,
        ):
            # per-ob bias ([128,512] slices) and c ([128,2,128] per m-tile)
            # land right before their o-block — keeps 24KB/partition free
            # so the weight-slab pool can triple-buffer
            obres = {}

            def load_ob_resources(ob):
                brt = constp.tile([128, NW], BF16, tag="bias_r", name="bias_r")
                bit = constp.tile([128, NW], BF16, tag="bias_i", name="bias_i")
                obw = slice(ob * NW, (ob + 1) * NW)
                nc.gpsimd.dma_start(out=brt[:], in_=bbc[0][:, obw])
                nc.gpsimd.dma_start(out=bit[:], in_=bbc[1][:, obw])
                cts = []
                for m in range(MT):
                    t = cresp.tile([128, 2, 128], BF16, tag=f"c_m{m}",
                                   name=f"c_m{m}")
                    src = cx[m * 128 : (m + 1) * 128, :].rearrange(
                        "r (two h) -> r two h", two=2
                    )[:, :, ob * 128 : (ob + 1) * 128]
                    nc.gpsimd.dma_start(out=t[:], in_=src)
                    cts.append(t)
                obres[ob] = (brt, bit, cts)

            # X tiles: one [128, KT, BL] tile per tensor; DR slices
            # [:, 2k:2k+2, m*128:(m+1)*128] need k-pairs adjacent in dim1.
            xht = [xtp.tile([128, KT, BL], F8, tag=f"xh{m}", name=f"xh{m}")
                   for m in range(3)]
            xlt = [xtp.tile([128, KT, BL], F8, tag=f"xl{m}", name=f"xl{m}")
                   for m in range(3)]
            att = [xtp.tile([128, KT, BL], F8, tag=f"xa{m}", name=f"xa{m}")
                   for m in range(3)]

            def emit_xloads(src, dst, quarters=4, first_pair_fast=False):
                # dram [KT,128,BL] -> sbuf [128, KT, BL] in `quarters` DMAs;
                # first_pair_fast peels k-tiles 0-1 onto the idle DVE HWDGE
                # ring so the very first matmul unblocks sooner (ACT is idle
                # at t=0; DVE cannot issue DMAs in this build).
                dmas = []
                kq = KT // quarters
                for q in range(quarters):
                    eng = nc.scalar if first_pair_fast else nc.gpsimd
                    if first_pair_fast and q == 0:
                        # peel k-tiles 0-1 so the first matmul unblocks at
                        # the DGE-init floor
                        dmas.append(nc.scalar.dma_start(
                            out=dst[:, 0:2, :],
                            in_=src[0:2].rearrange("kt p b -> p kt b"),
                        ))
                        dmas.append(nc.scalar.dma_start(
                            out=dst[:, 2:4, :],
                            in_=src[2:4].rearrange("kt p b -> p kt b"),
                        ))
                        continue
                    dmas.append(eng.dma_start(
                        out=dst[:, q * kq : (q + 1) * kq, :],
                        in_=src[q * kq : (q + 1) * kq].rearrange(
                            "kt p b -> p kt b"
                        ),
                    ))
                return dmas

            p1s_all = {ob: [None] * MT for ob in range(OB)}
            p2s_all = {ob: [None] * MT for ob in range(OB)}
            pa = [None] * MT

            def emit_mat(ob, mat, ms=None, wtiles=None, split_ag=False,
                         defer_corr=False):
                ocols = slice(ob * 128, (ob + 1) * 128)
                oicols = slice(H + ob * 128, H + (ob + 1) * 128)
                p1s = p1s_all[ob]
                p2s = p2s_all[ob]
                if ms is None:
                    ms = range(MT)
                if mat == 0 and ob not in obres:
                    load_ob_resources(ob)
                if wtiles is None:
                    wtiles = load_slabs(ob, mat)
                return emit_mat_body(ob, mat, ms, wtiles, ocols, oicols,
                                     p1s, p2s, split_ag, defer_corr)

            def load_slabs(ob, mat):
                w = wp.tile([128, KT, 2 * NW], F8, tag="wslab", name="wslab")
                wsrc = wcomb[mat, ob].rearrange("p kt c -> p (kt c)")
                wdst = w.rearrange("p kt c -> p (kt c)")
                if ob == 0 and mat == 0:
                    # chunk the very first slab by k-pairs so every pass
                    # type starts as soon as its k-tiles land
                    qr = KT * 2 * NW // 8
                    for _q in range(8):
                        nc.sync.dma_start(
                            out=wdst[:, _q * qr : (_q + 1) * qr],
                            in_=wsrc[:, _q * qr : (_q + 1) * qr],
                        )
                elif ob == 0 and mat == 1:
                    half = KT * 2 * NW // 2
                    nc.sync.dma_start(out=wdst[:, :half], in_=wsrc[:, :half])
                    nc.sync.dma_start(out=wdst[:, half:], in_=wsrc[:, half:])
                else:
                    nc.sync.dma_start(out=wdst[:], in_=wsrc)
                return w

            def emit_mat_body(ob, mat, ms, wtiles, ocols, oicols,
                              p1s, p2s, split_ag=False, defer_corr=False):
                w = wtiles
                me = MIX_END[mat]

                def emit_group(ps, rows, c0, c1, mat, phase="all"):
                    # one accumulation group covering columns [c0:c1]:
                    # main everywhere, mixed pass on [c0:me), full hi/lo
                    # passes on [me:c1). phase="main"/"corr" emits a
                    # subset (used while the slab is still streaming in).
                    passes = [("m", c0, c1)]
                    if c1 > me:
                        passes.append(("x", max(c0, me), c1))
                        passes.append(("w", max(c0, me), c1))
                    if c0 < me:
                        passes.append(("a", c0, min(c1, me)))
                    if phase == "main":
                        passes = passes[:1]
                    elif phase == "corr":
                        passes = [("_skip", c0, c1)] + passes[1:]
                    for ci, (side, d0, d1) in enumerate(passes):
                        if side == "_skip":
                            continue
                        for k in range(0, KT, 2):
                            if side == "m":
                                lhsT = xht[mat][:, k : k + 2, rows]
                                rhs = w[:, k : k + 2, d0:d1]
                            elif side == "a":
                                lhsT = att[mat][:, k : k + 2, rows]
                                rhs = w[:, k : k + 2, NW + d0 : NW + d1]
                            elif side == "x":
                                lhsT = xlt[mat][:, k : k + 2, rows]
                                rhs = w[:, k : k + 2, d0:d1]
                            else:
                                lhsT = xht[mat][:, k : k + 2, rows]
                                rhs = w[:, k : k + 2, NW + d0 : NW + d1]
                            nc.tensor.matmul(
                                ps[:, d0:d1],
                                lhsT=lhsT,
                                rhs=rhs,
                                start=(ci == 0 and k == 0),
                                stop=(phase != "main" and
                                      ci == len(passes) - 1 and k == KT - 2),
                                perf_mode=DR,
                            )

                deferred = {}
                if defer_corr:
                    for m in ms:
                        rows = slice(m * 128, (m + 1) * 128)
                        ps = psmm.tile([128, NW], F32, tag="mm", name="mm")
                        emit_group(ps, rows, 0, NW, mat, phase="main")
                        deferred[m] = ps
                for m in ms:
                    rows = slice(m * 128, (m + 1) * 128)
                    if m in deferred:
                        ps = deferred[m]
                        emit_group(ps, rows, 0, NW, mat, phase="corr")
                    elif split_ag and mat == 2:
                        # tail latency trick: a-gate columns get their own
                        # PSUM group that completes one pass-set early, so
                        # the tanh (longest downstream pole) runs while the
                        # sigmoid columns still accumulate
                        psA = psmm.tile([128, NW], F32, tag="mm", name="mm")
                        emit_group(psA, rows, 384, NW, mat)
                        ps = psmm.tile([128, NW], F32, tag="mm", name="mm")
                        emit_group(ps, rows, 0, 384, mat)
                        pa[m] = pa[m] + (psA,)
                    else:
                        ps = psmm.tile([128, NW], F32, tag="mm", name="mm")
                        emit_group(ps, rows, 0, NW, mat)

                    if mat == 0:
                        p1s[m] = pp.tile([128, NW], F32, tag=f"p1_{m}",
                                         name=f"p1_{m}")
                        nc.scalar.copy(p1s[m][:], ps[:])
                    elif mat == 1:
                        p2s[m] = pp.tile([128, NW], F32, tag=f"p2_{m}",
                                         name=f"p2_{m}")
                        nc.scalar.copy(p2s[m][:], ps[:])
                        # ---- epilogue phase A: everything that only
                        # needs P1/P2 (not P3) — overlaps the P3 matmuls.
                        # Gate cols: o=[0:128] f=[128:256] i=[256:384]
                        # a=[384:512]. PSUM holds SW*z.
                        brt, bit, cts = obres[ob]
                        p1, p2 = p1s[m], p2s[m]
                        zr = epp.tile([128, NW], F32, tag="zra", name="zra")
                        nc.vector.tensor_sub(zr[:], p1[:], p2[:])
                        nc.gpsimd.tensor_add(zr[:], zr[:], brt[:])
                        gr = epp.tile([128, NW], F32, tag=f"gr_{m}",
                                      name=f"gr_{m}", bufs=1)
                        nc.scalar.activation(gr[:, 0:384], zr[:, 0:384],
                                             AFT.Sigmoid, scale=1.0 / SP)
                        nc.scalar.activation(gr[:, 384:512], zr[:, 384:512],
                                             AFT.Tanh, scale=1.0 / SP)
                        # q = p1 + p2 - bias_i (scaled): phase B does
                        # zi = P3 - q in one DVE op off the bias path
                        q = epp.tile([128, NW], F32, tag=f"q_{m}",
                                     name=f"q_{m}", bufs=1)
                        nc.vector.tensor_add(q[:], p1[:], p2[:])
                        nc.gpsimd.tensor_sub(q[:], q[:], bit[:])
                        cr = cts[m][:, 0, :]
                        ci = cts[m][:, 1, :]
                        ir_ = gr[:, 0:128]
                        fr = gr[:, 128:256]
                        orr = gr[:, 256:384]
                        ar = gr[:, 384:512]
                        u1 = prodp.tile([128, 128], F32, tag=f"u1_{m}",
                                        name=f"u1_{m}", bufs=1)
                        u4 = prodp.tile([128, 128], F32, tag=f"u4_{m}",
                                        name=f"u4_{m}", bufs=1)
                        v1 = prodp.tile([128, 128], F32, tag=f"v1_{m}",
                                        name=f"v1_{m}", bufs=1)
                        nc.vector.tensor_mul(u1[:], cr, fr)
                        nc.vector.tensor_mul(u4[:], ci, fr)
                        nc.vector.tensor_mul(v1[:], ar, ir_)
                        pa[m] = (gr, q, u1, u4, v1)
                    else:
                        # ---- epilogue phase B for (ob, m): ps holds P3 ----
                        if split_ag:
                            gr, q, u1, u4, v1, psA = pa[m]
                        else:
                            gr, q, u1, u4, v1 = pa[m]
                            psA = ps
                        zi = epp.tile([128, NW], F32, tag="zi", name="zi")
                        gi = epp.tile([128, NW], F32, tag="gi", name="gi")
                        # a-gate first: with split_ag its group closed while
                        # the sigmoid columns still accumulate, and its tanh
                        # is the longest downstream dependency
                        nc.vector.tensor_sub(zi[:, 384:512], psA[:, 384:512],
                                             q[:, 384:512])
                        nc.scalar.activation(gi[:, 384:512], zi[:, 384:512],
                                             AFT.Tanh, scale=1.0 / SP)
                        nc.vector.tensor_sub(zi[:, 0:384], ps[:, 0:384],
                                             q[:, 0:384])
                        nc.scalar.activation(gi[:, 0:384], zi[:, 0:384],
                                             AFT.Sigmoid, scale=1.0 / SP)
                        cr = obres[ob][2][m][:, 0, :]
                        ci = obres[ob][2][m][:, 1, :]
                        ii_ = gi[:, 0:128]
                        fi = gi[:, 128:256]
                        oi = gi[:, 256:384]
                        ai = gi[:, 384:512]
                        ir_ = gr[:, 0:128]
                        orr = gr[:, 256:384]
                        ar = gr[:, 384:512]
                        u2 = prodp.tile([128, 128], F32, tag="u2", name="u2", bufs=1)
                        u3 = prodp.tile([128, 128], F32, tag="u3", name="u3", bufs=1)
                        v2 = prodp.tile([128, 128], F32, tag="v2", name="v2", bufs=1)
                        v3 = prodp.tile([128, 128], F32, tag="v3", name="v3", bufs=1)
                        v4 = prodp.tile([128, 128], F32, tag="v4", name="v4", bufs=1)
                        nc.vector.tensor_mul(u2[:], ci, fi)
                        nc.vector.tensor_mul(u3[:], cr, fi)
                        nc.gpsimd.tensor_mul(v2[:], ai, ii_)
                        nc.gpsimd.tensor_mul(v3[:], ar, ii_)
                        nc.vector.tensor_mul(v4[:], ai, ir_)
                        cfr = prodp.tile([128, 128], F32, tag="cfr", name="cfr", bufs=1)
                        cfi = prodp.tile([128, 128], F32, tag="cfi", name="cfi", bufs=1)
                        air = prodp.tile([128, 128], F32, tag="air", name="air", bufs=1)
                        aii = prodp.tile([128, 128], F32, tag="aii", name="aii", bufs=1)
                        nc.vector.tensor_sub(cfr[:], u1[:], u2[:])
                        nc.vector.tensor_add(cfi[:], u3[:], u4[:])
                        nc.gpsimd.tensor_sub(air[:], v1[:], v2[:])
                        nc.gpsimd.tensor_add(aii[:], v3[:], v4[:])
                        # c_t in bf16 (output dtype): tanh reads bf16 fine
                        ctr = prodp.tile([128, 128], BF16, tag="ctr", name="ctr")
                        cti = prodp.tile([128, 128], BF16, tag="cti", name="cti")
                        nc.vector.tensor_add(ctr[:], cfr[:], air[:])
                        nc.vector.tensor_add(cti[:], cfi[:], aii[:])
                        tr = prodp.tile([128, 128], F32, tag="tr", name="tr")
                        ti = prodp.tile([128, 128], F32, tag="ti", name="ti")
                        nc.scalar.activation(tr[:], ctr[:], AFT.Tanh)
                        nc.scalar.activation(ti[:], cti[:], AFT.Tanh)
                        htr = prodp.tile([128, 128], BF16, tag="htr", name="htr")
                        hti = prodp.tile([128, 128], BF16, tag="hti", name="hti")
                        w1 = prodp.tile([128, 128], F32, tag="w1", name="w1")
                        w2 = prodp.tile([128, 128], F32, tag="w2", name="w2")
                        w3 = prodp.tile([128, 128], F32, tag="w3", name="w3")
                        w4 = prodp.tile([128, 128], F32, tag="w4", name="w4")
                        # real half on DVE, imag half on GPSIMD in parallel
                        nc.vector.tensor_mul(w1[:], orr, tr[:])
                        nc.vector.tensor_mul(w2[:], oi, ti[:])
                        nc.vector.tensor_sub(htr[:], w1[:], w2[:])
                        nc.gpsimd.tensor_mul(w3[:], orr, ti[:])
                        nc.gpsimd.tensor_mul(w4[:], oi, tr[:])
                        nc.gpsimd.tensor_add(hti[:], w3[:], w4[:])
                        # keep the SP ring clean for the weight-slab
                        # stream: outputs ride Pool/ACT
                        nc.scalar.dma_start(out=h_out[rows, ocols], in_=htr[:])
                        nc.gpsimd.dma_start(out=h_out[rows, oicols], in_=hti[:])
                        nc.gpsimd.dma_start(out=c_out[rows, ocols], in_=ctr[:])
                        nc.scalar.dma_start(out=c_out[rows, oicols], in_=cti[:])

            # PE stream order: X1 hi/lo loads, then the first matmul block
            # (only needs X1), then X2/X3 loads while that block runs.
            d1 = emit_xloads(xh[0], xht[0], first_pair_fast=True)
            d1 += emit_xloads(xl[0], xlt[0])
            d1 += emit_xloads(xa[0], att[0])
            emit_mat(0, 0, defer_corr=True)
            d2 = []
            for m in (1, 2):
                d2 += emit_xloads(xh[m], xht[m])
                d2 += emit_xloads(xl[m], xlt[m])
                d2 += emit_xloads(xa[m], att[m])
            emit_mat(0, 1, defer_corr=True)
            emit_mat(0, 2)
            for ob in range(1, OB):
                for mat in range(3):
                    if ob == OB - 1 and mat == 2:
                        # split the final P3 so only two phase-B chains
                        # drain after the last matmul
                        wt = load_slabs(ob, mat)
                        emit_mat(ob, mat, ms=range(0, 2), wtiles=wt)
                        emit_mat(ob, mat, ms=range(2, MT), wtiles=wt,
                                 split_ag=True)
                    else:
                        emit_mat(ob, mat)
    return nc


_NC_CACHE = None


def _get_program():
    global _NC_CACHE
    if _NC_CACHE is None:
        nc = _build_program()
        fixed = _split_multiwait_json(nc.to_json_bytes())
        nc.to_json_bytes = lambda: fixed
        _NC_CACHE = nc
    return _NC_CACHE


F8NP = ml_dtypes.float8_e4m3


def _split8(a):
    a = a * SX
    ah = a.astype(F8NP)
    ahf = ah.astype(np.float32)
    al = (a - ahf).astype(F8NP)
    aa = (0.5 * ahf + al.astype(np.float32)).astype(F8NP)
    return ah, al, aa


def _pack_weights(Uw_r, Uw_i, Ub_r, Ub_i, Ww_r, Ww_i, Wb_r, Wb_i):
    GORD = [1, 0, 3, 2]  # column blocks [i, f, o, a]: sigmoid trio is
    # [0:384] (one act call), tanh at [384:512]; correction suffixes cover
    # gates in sensitivity order a > o > f > i.

    def interleave_cols(Wg):  # [2048, G, H] -> [2048, GH]
        return (
            Wg.reshape(K, G, OB, 128)[:, GORD]
            .transpose(0, 2, 1, 3)
            .reshape(K, G * H)
        )

    Wr = np.concatenate(
        [np.transpose(Uw_r, (2, 0, 1)), np.transpose(Ww_r, (2, 0, 1))], axis=0
    )
    Wi = np.concatenate(
        [np.transpose(Uw_i, (2, 0, 1)), np.transpose(Ww_i, (2, 0, 1))], axis=0
    )
    W1 = interleave_cols(Wr) * SW
    W2 = interleave_cols(Wi) * SW
    W3 = W1 + W2
    Wall = np.stack([W1, W2, W3])  # [3, 2048, 4096] f32, pre-scaled

    def slabify(Wm, cols):  # [2048, ncols] -> [ob, 128, KT, ncols_per_ob]
        ncpo = cols
        return (
            Wm.reshape(KT, 128, OB, ncpo)
            .transpose(2, 1, 0, 3)
        )

    whs = []
    wls = []
    bss = []
    for m in range(3):
        me = MIX_END[m]
        Wh8 = Wall[m].astype(F8NP)
        Whf = Wh8.astype(np.float32)
        Wl8 = (Wall[m] - Whf).astype(F8NP)
        B8 = (Whf + 2.0 * Wl8.astype(np.float32)).astype(F8NP)
        # halve the mixed-prefix columns of Wh (exact: exponent decrement)
        Whm = Whf.reshape(K, OB, NW)
        Whm[:, :, :me] *= 0.5
        Wh8m = Whm.reshape(K, G * H).astype(F8NP)
        whm_slab = Wh8m.reshape(KT, 128, OB, NW).transpose(2, 1, 0, 3)
        b_slab = B8.reshape(KT, 128, OB, NW).transpose(2, 1, 0, 3)[:, :, :, :me]
        l_slab = Wl8.reshape(KT, 128, OB, NW).transpose(2, 1, 0, 3)[:, :, :, me:]
        whs.append(np.concatenate([whm_slab, b_slab, l_slab], axis=-1))
    wh = np.ascontiguousarray(np.stack(whs))  # [3, OB, 128, KT, 2*NW]

    def interleave_bias(b):  # [G, H] -> [GH] interleaved, pre-scaled
        return b.reshape(G, OB, 128)[GORD].transpose(1, 0, 2).reshape(G * H)

    br = interleave_bias((Ub_r + Wb_r) * SP)
    bi = interleave_bias((Ub_i + Wb_i) * SP)
    bbc = np.ascontiguousarray(np.broadcast_to(
        np.stack([br, bi])[:, None, :], (2, 128, G * H)
    ).astype(ml_dtypes.bfloat16))
    return wh, bbc


def kernel(input, h_x, c_x, Uw_r, Uw_i, Ub_r, Ub_i, Ww_r, Ww_i, Wb_r, Wb_i,
           _trace=False):
    input = np.asarray(input, dtype=np.float32)
    h_x = np.asarray(h_x, dtype=np.float32)
    c_x = np.asarray(c_x, dtype=np.float32)
    wh, bpk = _pack_weights(
        np.asarray(Uw_r, np.float32), np.asarray(Uw_i, np.float32),
        np.asarray(Ub_r, np.float32), np.asarray(Ub_i, np.float32),
        np.asarray(Ww_r, np.float32), np.asarray(Ww_i, np.float32),
        np.asarray(Wb_r, np.float32), np.asarray(Wb_i, np.float32),
    )

    X1 = np.concatenate([input[:, :IN], h_x[:, :H]], axis=1)
    X2 = np.concatenate([input[:, IN:], h_x[:, H:]], axis=1)
    X3 = X1 + X2
    xparts = [_split8(X) for X in (X1, X2, X3)]

    in_maps = []
    for c in range(NCORES):
        rows = slice(c * BL, (c + 1) * BL)
        im = {
            "cx": np.ascontiguousarray(c_x[rows].astype(ml_dtypes.bfloat16)),
            "wcomb": wh,
            "bbc": bpk,
        }
        for m in range(3):
            xhm, xlm, xam = xparts[m]
            im[f"x{m}h"] = np.ascontiguousarray(
                xhm[rows].T.reshape(KT, 128, BL)
            )
            im[f"x{m}l"] = np.ascontiguousarray(
                xlm[rows].T.reshape(KT, 128, BL)
            )
            im[f"x{m}a"] = np.ascontiguousarray(
                xam[rows].T.reshape(KT, 128, BL)
            )
        in_maps.append(im)

    nc = _get_program()
    res = run_bass_kernel_spmd(
        nc, in_maps, core_ids=list(range(NCORES)), trace=_trace
    )
    h_t = np.concatenate(
        [res.results[i]["h_out"].astype(np.float32) for i in range(NCORES)],
        axis=0,
    )
    c_t = np.concatenate(
        [res.results[i]["c_out"].astype(np.float32) for i in range(NCORES)],
        axis=0,
    )
    if _trace:
        kernel._last_results = res
    return h_t, c_t
